# revision 1
# baseline (speedup 1.0000x reference)
"""Trainium2 Bass kernel for nn_DSR_GCN (dual-superpixel GCN).

Sharding (8 NeuronCores, SPMD): row-shard the HW=65536 pixel dim (8192
rows/core).  Pass 1 computes per-core partials G.T = x_shard.T @ Q_shard
(column sums of Q are precomputed on host and folded in after the
AllReduce).  The small [N,N] GCN math is replicated per core in
feature-major layout.  d = rowsum(S*A)+1 is accumulated as a
128-row broadcast via an all-ones stationary so the rsqrt pipeline runs
as full-partition elementwise ops (no [1,n] single-lane work).  Pass 2
computes z.T = RP1.T @ Q.T + RP2.T @ Qs.T with the final linears folded
into [N,32] bf16 stationaries; y-feature linear is folded with its bias
via an appended ones-row.  Heavy matmul streams are bf16 (host-cast).
"""

import os
import numpy as np
import ml_dtypes

BF16 = ml_dtypes.bfloat16

HW, C = 65536, 128
NB, NS, NCLS = 1024, 2048, 16
NCORES = 8
EPS = 1e-5
CLAMP = 0.03
FOS = [128, 64, 128, 64]


def _emit(nc, tc, ctx, rows, nb, ns, ncores):
    import concourse.bass as bass
    import concourse.mybir as mybir
    from concourse import masks
    from contextlib import ExitStack

    f32 = mybir.dt.float32
    bf16 = mybir.dt.bfloat16
    ts = bass.ts
    AF = mybir.ActivationFunctionType
    ALU = mybir.AluOpType
    AX = mybir.AxisListType.X

    # ---- dram I/O ----
    din = lambda n_, s, d: nc.dram_tensor(n_, s, d, kind="ExternalInput")
    xs = din("xs", [rows, C], bf16)
    q = din("q", [rows, nb], bf16)
    qs = din("qs", [rows, ns], bf16)
    qbt = din("qbt", [nb, rows], bf16)
    qst = din("qst", [ns, rows], bf16)
    at = din("at", [nb, nb], bf16)
    ast = din("ast", [ns, ns], bf16)
    yte = din("yte", [65, rows], bf16)
    icsb = din("icsb", [1, nb], bf16)
    icss = din("icss", [1, ns], bf16)
    w128e = din("w128e", [65, 16], bf16)
    wls = [din(f"wl{i}", [128, 256 + 2 * fo + 5], f32) for i, fo in enumerate(FOS)]
    misc = din("misc", [64, 64], f32)
    yo = nc.dram_tensor("yo", [rows, NCLS], f32, kind="ExternalOutput")
    lo = nc.dram_tensor("lo", [rows, NCLS], f32, kind="ExternalOutput")

    # ---- persistent pools ----
    consts = ctx.enter_context(tc.tile_pool(name="consts", bufs=1))
    gwork = ctx.enter_context(tc.tile_pool(name="gwork", bufs=1))
    dram = ctx.enter_context(tc.tile_pool(name="dram", bufs=1, space="DRAM"))

    ident32 = consts.tile([32, 32], f32)
    masks.make_identity(nc, ident32[:])
    ident1 = consts.tile([1, 1], f32)
    nc.gpsimd.memset(ident1[:], 1.0)
    ones_k1 = consts.tile([1, 128], bf16)
    nc.gpsimd.memset(ones_k1[:], 1.0)
    ones_bf = consts.tile([128, 128], bf16)
    nc.gpsimd.memset(ones_bf[:], 1.0)
    one_col = consts.tile([128, 1], f32)
    nc.gpsimd.memset(one_col[:], 1.0)
    eps_c = consts.tile([128, 1], f32)
    nc.gpsimd.memset(eps_c[:], EPS)
    thr03 = consts.tile([128, 1], f32)
    nc.gpsimd.memset(thr03[:], CLAMP)

    misc_sb = consts.tile([64, 64], f32)
    nc.sync.dma_start(misc_sb[:], misc[:])
    w128_sb = consts.tile([65, 16], bf16)
    nc.sync.dma_start(w128_sb[:], w128e[:])
    icsb_sb = consts.tile([1, nb], bf16)
    nc.sync.dma_start(icsb_sb[:], icsb[:])
    icss_sb = consts.tile([1, ns], bf16)
    nc.sync.dma_start(icss_sb[:], icss[:])
    wl_sb = []
    for i, fo in enumerate(FOS):
        t = consts.tile([128, 256 + 2 * fo + 5], f32, tag=f"wl{i}")
        nc.sync.dma_start(t[:], wls[i][:])
        wl_sb.append(t)
    wcb_bf = consts.tile([64, 32], bf16)
    nc.vector.tensor_copy(wcb_bf[:], misc_sb[:, 0:32])
    wcs_bf = consts.tile([64, 32], bf16)
    nc.vector.tensor_copy(wcs_bf[:], misc_sb[:, 32:64])

    # ---- pass 1 (no colsum matmuls: 1/colsum comes from host) ----
    n_rt = rows // 128
    shkw = {"addr_space": "Shared"} if ncores > 4 else {}
    ar1_in = dram.tile([128, nb], f32, tag="ar1i")
    ar1_out = dram.tile([128, nb], f32, tag="ar1o", **shkw)
    ar2_in = dram.tile([128, ns], f32, tag="ar2i")
    ar2_out = dram.tile([128, ns], f32, tag="ar2o", **shkw)

    with tc.tile_pool(name="p1pool", bufs=1) as p1pool:
        xall = p1pool.tile([128, n_rt * C], bf16, tag="xall")
        xcnk = n_rt // 4
        for xc in range(4):
            nc.sync.dma_start(
                xall[:, xc * xcnk * C:(xc + 1) * xcnk * C].rearrange(
                    "p (t c) -> p t c", c=C),
                xs[xc * xcnk * 128:(xc + 1) * xcnk * 128, :].rearrange(
                    "(t p) c -> p t c", p=128))

        def pass1_phase(qd, n, g_ps, rgrp, qtag, qpool):
            for g in range(n_rt // rgrp):
                qt = qpool.tile([128, rgrp * n], bf16, tag=qtag)
                for a in range(rgrp):
                    rt = g * rgrp + a
                    nc.gpsimd.dma_start(qt[:, a * n:(a + 1) * n],
                                        qd[rt * 128:(rt + 1) * 128, :])
                for a in range(rgrp):
                    rt = g * rgrp + a
                    xt = xall[:, ts(rt, C)]
                    st = (rt == 0)
                    sp = (rt == n_rt - 1)
                    for cnk in range(n // 512):
                        mv = qt[:, a * n + cnk * 512:a * n + (cnk + 1) * 512]
                        nc.tensor.matmul(g_ps[:, ts(cnk, 512)], xt, mv,
                                         start=st, stop=sp)

        with tc.tile_pool(name="ps_p1b", bufs=1, space="PSUM") as psb, \
             tc.tile_pool(name="qpb", bufs=4) as qpool:
            g1p = psb.tile([128, nb], f32, tag="g1p")
            pass1_phase(q, nb, g1p, min(4096 // nb, n_rt), "qb", qpool)
            g1t = p1pool.tile([128, nb], f32, tag="g1t")
            nc.vector.tensor_copy(g1t[:], g1p[:])

        # big-branch AllReduce early: overlaps small pass-1
        nc.sync.dma_start(ar1_in[:], g1t[:])
        nc.gpsimd.collective_compute(
            "AllReduce", mybir.AluOpType.add,
            replica_groups=[list(range(ncores))],
            ins=[ar1_in.opt()], outs=[ar1_out.opt()])

        with tc.tile_pool(name="ps_p1s", bufs=1, space="PSUM") as pss, \
             tc.tile_pool(name="qps", bufs=4) as qpool:
            g2p = pss.tile([128, ns], f32, tag="g2p")
            pass1_phase(qs, ns, g2p, min(4096 // ns, n_rt), "qs", qpool)
            g2t = p1pool.tile([128, ns], f32, tag="g2t")
            nc.vector.tensor_copy(g2t[:], g2p[:])

        nc.sync.dma_start(ar2_in[:], g2t[:])
        nc.gpsimd.collective_compute(
            "AllReduce", mybir.AluOpType.add,
            replica_groups=[list(range(ncores))],
            ins=[ar2_in.opt()], outs=[ar2_out.opt()])

    # ---- pass-2 prefetch: big-branch Q.T tiles for group 0 during GCN ----
    GRP = min(2048, rows)
    p2pre = ctx.enter_context(tc.tile_pool(name="p2pre", bufs=1))
    pre_qb = []
    for j in range(nb // 128):
        t = p2pre.tile([128, GRP], bf16, tag=f"pre{j}")
        nc.gpsimd.dma_start(t[:], qbt[ts(j, 128), 0:GRP])
        pre_qb.append(t)

    # ---- GCN (replicated per core) ----
    def gcn_branch(n, ar_out, at_d, ics_sb, lidx, clamp, hfin):
        njt = n // 128
        ncnk = n // 512
        with ExitStack() as bctx:
            bp = bctx.enter_context(tc.tile_pool(name=f"b_{n}", bufs=1))

            # H = G * (1/colsum) ; broadcast 1/colsum across partitions via
            # K=1 ones matmul
            ht = bp.tile([128, n], f32, tag="hcur0")
            with tc.tile_pool(name=f"psr_{n}", bufs=2, space="PSUM") as psr, \
                 tc.tile_pool(name=f"icsp_{n}", bufs=1) as icsp:
                g_sb = icsp.tile([128, n], f32, tag="g_sb")
                nc.gpsimd.dma_start(g_sb[:], ar_out[:])
                for cnk in range(ncnk):
                    pr = psr.tile([128, 512], f32)
                    nc.tensor.matmul(pr[:], ones_k1[:],
                                     ics_sb[:, ts(cnk, 512)],
                                     start=True, stop=True)
                    nc.vector.tensor_tensor(
                        ht[:, ts(cnk, 512)], g_sb[:, ts(cnk, 512)], pr[:],
                        op=ALU.mult)

            for li2, wli in enumerate(lidx):
                fo = FOS[wli]
                wl = wl_sb[wli]
                last = (li2 == 1)
                c0 = 256 + fo
                thWT = wl[:, 0:256]
                oWT = wl[:, 256:256 + fo]
                bng = wl[:, c0:c0 + 1]
                bnb = wl[:, c0 + 1:c0 + 2]
                thb = [wl[:, c0 + 2:c0 + 3], wl[:, c0 + 3:c0 + 4]]
                ob_col = wl[0:fo, c0 + 4:c0 + 5]
                ob_rep = wl[:, c0 + 5:c0 + 5 + fo]  # host-replicated rows

                with ExitStack() as lctx:
                    lp = lctx.enter_context(
                        tc.tile_pool(name=f"l_{n}{li2}", bufs=1))
                    sp = lctx.enter_context(
                        tc.tile_pool(name=f"sp_{n}{li2}", bufs=2))

                    # --- batchnorm over nodes (free dim) ---
                    s1 = sp.tile([128, 1], f32, tag="s1")
                    nc.vector.reduce_sum(out=s1[:], in_=ht[:], axis=AX)
                    s2p = sp.tile([128, ncnk], f32, tag="s2p")
                    sqs = sp.tile([128, 512], bf16, tag="sqscratch")
                    for cnk in range(ncnk):
                        nc.scalar.activation(
                            sqs[:], ht[:, ts(cnk, 512)], AF.Square,
                            accum_out=s2p[:, cnk:cnk + 1])
                    s2 = sp.tile([128, 1], f32, tag="s2")
                    nc.vector.reduce_sum(out=s2[:], in_=s2p[:], axis=AX)
                    m = sp.tile([128, 1], f32, tag="m")
                    nc.vector.tensor_scalar_mul(m[:], s1[:], 1.0 / n)
                    v = sp.tile([128, 1], f32, tag="v")
                    nc.vector.tensor_scalar_mul(v[:], s2[:], 1.0 / n)
                    m2 = sp.tile([128, 1], f32, tag="m2")
                    nc.vector.tensor_tensor(m2[:], m[:], m[:], op=ALU.mult)
                    nc.vector.tensor_tensor(v[:], v[:], m2[:], op=ALU.subtract)
                    sd = sp.tile([128, 1], f32, tag="sd")
                    nc.scalar.activation(sd[:], v[:], AF.Sqrt, bias=eps_c[:])
                    isd = sp.tile([128, 1], f32, tag="isd")
                    nc.vector.reciprocal(isd[:], sd[:])
                    kk = sp.tile([128, 1], f32, tag="kk")
                    nc.vector.tensor_tensor(kk[:], bng, isd[:], op=ALU.mult)
                    b2 = sp.tile([128, 1], f32, tag="b2")
                    nc.vector.tensor_tensor(b2[:], m[:], kk[:], op=ALU.mult)
                    nc.vector.tensor_tensor(b2[:], bnb, b2[:], op=ALU.subtract)
                    hbn = lp.tile([128, n], bf16, tag="hbn")
                    nc.vector.tensor_scalar(hbn[:], ht[:], kk[:], b2[:],
                                            op0=ALU.mult, op1=ALU.add)
                    thWT_bf = lp.tile([128, 256], bf16, tag="thWT_bf")
                    nc.vector.tensor_copy(thWT_bf[:], thWT)
                    oWT_bf = lp.tile([128, fo], bf16, tag="oWT_bf")
                    nc.vector.tensor_copy(oWT_bf[:], oWT)

                    # --- Hx.T = thW @ Hbn.T + thb ---
                    hx = [lp.tile([128, n], bf16, tag=f"hx{k}", name=f"hx{k}")
                          for k in range(2)]
                    with tc.tile_pool(name=f"psx_{n}{li2}", bufs=3,
                                      space="PSUM") as psx:
                        for k in range(2):
                            for cnk in range(ncnk):
                                px = psx.tile([128, 512], f32)
                                nc.tensor.matmul(
                                    px[:], thWT_bf[:, ts(k, 128)],
                                    hbn[:, ts(cnk, 512)],
                                    start=True, stop=True)
                                nc.vector.tensor_scalar_add(
                                    hx[k][:, ts(cnk, 512)], px[:], thb[k])

                    # --- S blocks -> sigmoid -> t = S'*A.T (clamp fused) ---
                    tt = []
                    with tc.tile_pool(name=f"pss_{n}{li2}", bufs=2,
                                      space="PSUM") as pssb, \
                         tc.tile_pool(name=f"atp_{n}{li2}", bufs=2) as atp:
                        for j in range(njt):
                            att = atp.tile([128, n], bf16)
                            nc.sync.dma_start(att[:], at_d[ts(j, 128), :])
                            px = pssb.tile([128, n], f32, tag="spsum")
                            for cnk in range(ncnk):
                                nc.tensor.matmul(px[:, ts(cnk, 512)],
                                                 hx[0][:, ts(j, 128)],
                                                 hx[0][:, ts(cnk, 512)],
                                                 start=True, stop=False)
                                nc.tensor.matmul(px[:, ts(cnk, 512)],
                                                 hx[1][:, ts(j, 128)],
                                                 hx[1][:, ts(cnk, 512)],
                                                 start=False, stop=True)
                            sbl = sp.tile([128, n], bf16, tag="sblk")
                            nc.scalar.activation(sbl[:], px[:], AF.Sigmoid)
                            tj = lp.tile([128, n], bf16, tag=f"tj{j}")
                            if clamp:
                                nc.vector.scalar_tensor_tensor(
                                    tj[:], sbl[:], thr03[:], att[:],
                                    op0=ALU.max, op1=ALU.mult)
                            else:
                                nc.vector.tensor_tensor(tj[:], sbl[:], att[:],
                                                        op=ALU.mult)
                            tt.append(tj)

                    # V_j = HoW_j + ob (independent of d; fills PE early)
                    stats = []
                    with tc.tile_pool(name=f"psh_{n}{li2}", bufs=3,
                                      space="PSUM") as psh:
                        for j in range(njt):
                            ph = psh.tile([128, fo], f32)
                            nc.tensor.matmul(ph[:], hbn[:, ts(j, 128)],
                                             oWT_bf[:], start=True, stop=True)
                            stj = lp.tile([128, fo], bf16, tag=f"st{j}")
                            nc.vector.tensor_tensor(stj[:], ph[:], ob_rep,
                                                    op=ALU.add)
                            stats.append(stj)

                    # --- d: 128-row broadcast colsum of t, then rsqrt;
                    # u matmuls run while sqrt/recip resolve (parked psum) ---
                    pr_inv = lp.tile([128, n], f32, tag="pr_inv")
                    zbuf = lp.tile([128, n], f32, tag="zbuf")
                    dcol = sp.tile([128, njt], f32, tag="dcol")
                    with tc.tile_pool(name=f"psu_{n}{li2}", bufs=ncnk,
                                      space="PSUM") as psu:
                        with tc.tile_pool(name=f"psd_{n}{li2}", bufs=1,
                                          space="PSUM") as psd:
                            dbc = psd.tile([128, n], f32, tag="dbc")
                            for j in range(njt):
                                for cnk in range(ncnk):
                                    nc.tensor.matmul(
                                        dbc[:, ts(cnk, 512)], ones_bf[:],
                                        tt[j][:, ts(cnk, 512)],
                                        start=(j == 0), stop=(j == njt - 1))
                            pus = []
                            for cnk in range(ncnk):
                                pu = psu.tile([fo, 512], f32)
                                nc.tensor.matmul(pu[:], oWT_bf[:],
                                                 hbn[:, ts(cnk, 512)],
                                                 start=True, stop=True)
                                pus.append(pu)
                            nc.scalar.activation(zbuf[:], dbc[:], AF.Sqrt,
                                                 bias=one_col[:])
                        nc.vector.reciprocal(pr_inv[:], zbuf[:])

                        # d as per-partition columns (PE transpose of one row)
                        with tc.tile_pool(name=f"pst_{n}{li2}", bufs=3,
                                          space="PSUM") as pst:
                            for j in range(njt):
                                pt = pst.tile([128, 1], f32, tag="dt")
                                nc.tensor.transpose(pt[:],
                                                    pr_inv[0:1, ts(j, 128)],
                                                    ident1[:])
                                nc.vector.tensor_copy(dcol[:, j:j + 1], pt[:])

                        # stat_j = d_j * V_j (in place), z1 = V.T * d
                        for j in range(njt):
                            nc.vector.tensor_scalar_mul(stats[j][:],
                                                        stats[j][:],
                                                        dcol[:, j:j + 1])
                        z1 = zbuf[0:fo, :]
                        for cnk in range(ncnk):
                            nc.vector.scalar_tensor_tensor(
                                z1[:, ts(cnk, 512)], pus[cnk][:], ob_col,
                                pr_inv[0:fo, ts(cnk, 512)],
                                op0=ALU.add, op1=ALU.mult)

                    # out.T = leaky(d * (stat.T @ t + z1))
                    hnext = hfin if last else bp.tile([128, n], f32,
                                                      tag="hcur1")
                    with tc.tile_pool(name=f"pso_{n}{li2}", bufs=1,
                                      space="PSUM") as pso:
                        po = pso.tile([fo, n], f32, tag="po")
                        for j in range(njt):
                            for cnk in range(ncnk):
                                nc.tensor.matmul(po[:, ts(cnk, 512)],
                                                 stats[j][:],
                                                 tt[j][:, ts(cnk, 512)],
                                                 start=(j == 0),
                                                 stop=(j == njt - 1))
                        for cnk in range(ncnk):
                            s2t = sp.tile([fo, 512], f32, tag="s2t")
                            nc.vector.tensor_tensor(
                                s2t[:], po[:, ts(cnk, 512)],
                                z1[:, ts(cnk, 512)], op=ALU.add)
                            s3t = sp.tile([fo, 512], f32, tag="s3t")
                            nc.vector.tensor_tensor(
                                s3t[:], s2t[:], pr_inv[0:fo, ts(cnk, 512)],
                                op=ALU.mult)
                            nc.scalar.activation(hnext[0:fo, ts(cnk, 512)],
                                                 s3t[:], AF.Lrelu, alpha=0.01)
                ht = hnext

    h1f = gwork.tile([64, nb], f32, tag="h1f")
    h2f = gwork.tile([64, ns], f32, tag="h2f")
    gcn_branch(nb, ar1_out, at, icsb_sb, [0, 1], True, h1f)
    gcn_branch(ns, ar2_out, ast, icss_sb, [2, 3], False, h2f)

    # RP stationaries [spix, 32] bf16, final linears folded
    h1b = gwork.tile([64, nb], bf16, tag="h1b")
    nc.vector.tensor_copy(h1b[:], h1f[:])
    h2b = gwork.tile([64, ns], bf16, tag="h2b")
    nc.vector.tensor_copy(h2b[:], h2f[:])
    rp1 = gwork.tile([128, (nb // 128) * 32], bf16, tag="rp1")
    rp2 = gwork.tile([128, (ns // 128) * 32], bf16, tag="rp2")
    with tc.tile_pool(name="psrp", bufs=3, space="PSUM") as psrp:
        for j in range(nb // 128):
            pr = psrp.tile([128, 32], f32)
            nc.tensor.matmul(pr[:], h1b[:, ts(j, 128)], wcb_bf[:],
                             start=True, stop=True)
            nc.vector.tensor_copy(rp1[:, ts(j, 32)], pr[:])
        for j in range(ns // 128):
            pr = psrp.tile([128, 32], f32)
            nc.tensor.matmul(pr[:], h2b[:, ts(j, 128)], wcs_bf[:],
                             start=True, stop=True)
            nc.vector.tensor_copy(rp2[:, ts(j, 32)], pr[:])

    # ---- pass 2 + epilogue ----
    nrc = max(GRP // 512, 1)
    CH = GRP // nrc
    with tc.tile_pool(name="qtp", bufs=6) as qtp, \
         tc.tile_pool(name="ytp", bufs=2) as ytp, \
         tc.tile_pool(name="ps_z", bufs=1, space="PSUM") as ps_z, \
         tc.tile_pool(name="ps_yw", bufs=2, space="PSUM") as ps_yw, \
         tc.tile_pool(name="ps_tp", bufs=2, space="PSUM") as ps_tp, \
         tc.tile_pool(name="epil", bufs=4) as ep:
        for gidx in range(rows // GRP):
            ytt = ytp.tile([65, GRP], bf16, tag="ytt")
            nc.sync.dma_start(ytt[:], yte[:, gidx * GRP:(gidx + 1) * GRP])
            pz = ps_z.tile([32, GRP], f32, tag="pz", name=f"pz{gidx}")
            for j in range(nb // 128):
                if gidx == 0:
                    tq = pre_qb[j]
                else:
                    tq = qtp.tile([128, GRP], bf16, tag="tqb")
                    nc.gpsimd.dma_start(
                        tq[:], qbt[ts(j, 128), gidx * GRP:(gidx + 1) * GRP])
                for rc in range(nrc):
                    nc.tensor.matmul(pz[:, ts(rc, CH)], rp1[:, ts(j, 32)],
                                     tq[:, ts(rc, CH)],
                                     start=(j == 0), stop=False)
            for j in range(ns // 128):
                tq = qtp.tile([128, GRP], bf16, tag="tqs")
                nc.gpsimd.dma_start(
                    tq[:], qst[ts(j, 128), gidx * GRP:(gidx + 1) * GRP])
                for rc in range(nrc):
                    nc.tensor.matmul(pz[:, ts(rc, CH)], rp2[:, ts(j, 32)],
                                     tq[:, ts(rc, CH)],
                                     start=False, stop=(j == ns // 128 - 1))
            for rc in range(nrc):
                base = gidx * GRP + rc * CH
                pyw = ps_yw.tile([16, CH], f32)
                nc.tensor.matmul(pyw[:], w128_sb[:], ytt[:, ts(rc, CH)],
                                 start=True, stop=True)
                yws = ep.tile([16, CH], f32, tag="yws")
                nc.scalar.activation(yws[:], pyw[:], AF.Copy)
                tri = ep.tile([32, CH], f32, tag="tri")
                nc.scalar.activation(tri[:], pz[:, ts(rc, CH)], AF.Copy)
                nc.vector.tensor_tensor(tri[0:16, :], pz[0:16, ts(rc, CH)],
                                        yws[:], op=ALU.add)
                for s in range(CH // 128):
                    ptr = ps_tp.tile([128, 32], f32)
                    nc.tensor.transpose(ptr[:], tri[:, ts(s, 128)], ident32[:])
                    mx = ep.tile([128, 1], f32, tag="mx")
                    nc.vector.reduce_max(out=mx[:], in_=ptr[:, 0:16], axis=AX)
                    nmx = ep.tile([128, 1], f32, tag="nmx")
                    nc.vector.tensor_scalar_mul(nmx[:], mx[:], -1.0)
                    e = ep.tile([128, 16], f32, tag="e")
                    ssum = ep.tile([128, 1], f32, tag="ssum")
                    nc.scalar.activation(e[:], ptr[:, 0:16], AF.Exp,
                                         bias=nmx[:], accum_out=ssum[:])
                    rcp = ep.tile([128, 1], f32, tag="rcp")
                    nc.vector.reciprocal(rcp[:], ssum[:])
                    yot = ep.tile([128, 16], f32, tag="yot")
                    nc.vector.tensor_scalar_mul(yot[:], e[:], rcp[:])
                    lot = ep.tile([128, 16], f32, tag="lot")
                    nc.scalar.activation(lot[:], ptr[:, 16:32], AF.Square)
                    nc.sync.dma_start(
                        yo[base + s * 128:base + (s + 1) * 128, :], yot[:])
                    nc.sync.dma_start(
                        lo[base + s * 128:base + (s + 1) * 128, :], lot[:])


def build(rows=HW // NCORES, nb=NB, ns=NS, ncores=NCORES):
    from contextlib import ExitStack
    import concourse.bacc as bacc
    import concourse.tile as tile

    nc = bacc.Bacc("TRN2", target_bir_lowering=False, debug=False,
                   enable_asserts=True, num_devices=ncores)
    with tile.TileContext(nc) as tc:
        with ExitStack() as ctx:
            _emit(nc, tc, ctx, rows, nb, ns, ncores)
    nc.compile()
    return nc


# --------------------------------------------------------------------------
# host wrapper
# --------------------------------------------------------------------------

def prep_inputs(rows, nb, ns, ncores,
                x, y, Q, A, Qsmall, Asmall,
                b0_bng, b0_bnb, b0_thW, b0_thb, b0_oW, b0_ob,
                b1_bng, b1_bnb, b1_thW, b1_thb, b1_oW, b1_ob,
                s0_bng, s0_bnb, s0_thW, s0_thb, s0_oW, s0_ob,
                s1_bng, s1_bnb, s1_thW, s1_thb, s1_oW, s1_ob,
                lin128_W, lin128_b, lin64_W, lin64_b, sigma2):
    f = np.float32
    hw = rows * ncores
    flat = np.ascontiguousarray(np.asarray(x, f).reshape(hw, -1))
    Q = np.asarray(Q, f)
    Qs = np.asarray(Qsmall, f)
    y = np.asarray(y, f)

    # 1/colsum of the bf16-cast Q (matches on-chip accumulation closely)
    icsb = (1.0 / Q.astype(BF16).astype(f).sum(axis=0))[None, :].astype(BF16)
    icss = (1.0 / Qs.astype(BF16).astype(f).sum(axis=0))[None, :].astype(BF16)

    def wl_pack(thW, thb, oW, ob, bng, bnb):
        fo = np.asarray(oW).shape[0]
        w = np.zeros((128, 256 + 2 * fo + 5), f)
        w[:, 0:256] = np.asarray(thW, f).T
        w[:, 256:256 + fo] = np.asarray(oW, f).T
        c0 = 256 + fo
        w[:, c0] = np.asarray(bng, f)
        w[:, c0 + 1] = np.asarray(bnb, f)
        w[:, c0 + 2] = np.asarray(thb, f)[0:128]
        w[:, c0 + 3] = np.asarray(thb, f)[128:256]
        w[0:fo, c0 + 4] = np.asarray(ob, f)
        w[:, c0 + 5:c0 + 5 + fo] = np.asarray(ob, f)[None, :]
        return w

    wl = [
        wl_pack(b0_thW, b0_thb, b0_oW, b0_ob, b0_bng, b0_bnb),
        wl_pack(b1_thW, b1_thb, b1_oW, b1_ob, b1_bng, b1_bnb),
        wl_pack(s0_thW, s0_thb, s0_oW, s0_ob, s0_bng, s0_bnb),
        wl_pack(s1_thW, s1_thb, s1_oW, s1_ob, s1_bng, s1_bnb),
    ]

    sig = float(np.asarray(sigma2).reshape(-1)[0])
    W128 = np.asarray(lin128_W, f)
    W64 = np.asarray(lin64_W, f)
    misc = np.zeros((64, 64), f)
    misc[:, 0:16] = sig * W128[:, :64].T
    misc[:, 16:32] = W64.T
    misc[:, 32:48] = (1.0 - sig) * W128[:, :64].T
    misc[:, 48:64] = -W64.T

    # y-linear with bias folded via appended ones-row
    w128e = np.zeros((65, 16), f)
    w128e[0:64, :] = W128[:, 64:].T
    w128e[64, :] = np.asarray(lin128_b, f)
    w128e = w128e.astype(BF16)

    at_b = np.ascontiguousarray(np.asarray(A, f).T).astype(BF16)
    ast_b = np.ascontiguousarray(np.asarray(Asmall, f).T).astype(BF16)

    in_maps = []
    for c in range(ncores):
        r0, r1 = c * rows, (c + 1) * rows
        qsh = Q[r0:r1]
        qssh = Qs[r0:r1]
        yte = np.ones((65, rows), f)
        yte[0:64, :] = y[r0:r1].T
        m = {
            "xs": flat[r0:r1].astype(BF16),
            "q": qsh.astype(BF16),
            "qs": qssh.astype(BF16),
            "qbt": np.ascontiguousarray(qsh.T).astype(BF16),
            "qst": np.ascontiguousarray(qssh.T).astype(BF16),
            "at": at_b,
            "ast": ast_b,
            "yte": yte.astype(BF16),
            "icsb": icsb,
            "icss": icss,
            "w128e": w128e,
            "misc": misc,
        }
        for i in range(4):
            m[f"wl{i}"] = wl[i]
        in_maps.append(m)
    return in_maps


_cache = {}
_last_results = None


def _ensure_ntff_hook():
    """Register the axon NTFF profile hook if the image's antenv lacks it."""
    import sys, types, ctypes, contextlib
    try:
        from antenv.axon_hooks import get_axon_ntff_profile_hook  # noqa: F401
        return True
    except ImportError:
        pass
    so_path = "/opt/axon/libaxon_pjrt.so"
    if not os.path.exists(so_path):
        return False
    lib = ctypes.CDLL(so_path)
    if not hasattr(lib, "axon_start_nrt_profile"):
        return False
    lib.axon_start_nrt_profile.argtypes = [ctypes.POINTER(ctypes.c_int64),
                                           ctypes.c_size_t]
    lib.axon_start_nrt_profile.restype = ctypes.c_int64
    lib.axon_stop_nrt_profile.argtypes = [ctypes.c_char_p]
    lib.axon_stop_nrt_profile.restype = ctypes.c_int64

    @contextlib.contextmanager
    def _hook(output_dir, device_ids):
        import jax
        jax.devices()
        if device_ids:
            ids = (ctypes.c_int64 * len(device_ids))(*device_ids)
            rc = lib.axon_start_nrt_profile(ids, len(device_ids))
        else:
            rc = lib.axon_start_nrt_profile(None, 0)
        if rc != 0:
            raise RuntimeError(f"axon_start_nrt_profile rc={rc}")
        try:
            yield
        finally:
            n = lib.axon_stop_nrt_profile(str(output_dir).encode())
            print(f"profile: {n} file(s) written to {output_dir}",
                  file=sys.stderr)

    mod = types.ModuleType("antenv.axon_hooks")
    holder = [_hook]
    mod.get_axon_ntff_profile_hook = lambda: holder[0]
    mod.set_axon_ntff_profile_hook = lambda h: holder.__setitem__(0, h)
    sys.modules["antenv.axon_hooks"] = mod
    import antenv
    antenv.axon_hooks = mod
    return True


def kernel(**inputs):
    global _last_results
    if "nc" not in _cache:
        _cache["nc"] = build()
    nc = _cache["nc"]
    rows = HW // NCORES
    in_maps = prep_inputs(rows, NB, NS, NCORES, **inputs)
    from concourse.bass_utils import run_bass_kernel_spmd
    trace = bool(os.environ.get("KERNEL_TRACE")) and _ensure_ntff_hook()
    res = run_bass_kernel_spmd(nc, in_maps, core_ids=list(range(NCORES)),
                               trace=trace)
    _last_results = res
    Y = np.concatenate([np.asarray(r["yo"]) for r in res.results], axis=0)
    L = np.concatenate([np.asarray(r["lo"]) for r in res.results], axis=0)
    return Y, L



# revision 21
# speedup vs baseline: 1.1900x; 1.1900x over previous
"""Trainium2 Bass kernel for nn_DSR_GCN (dual-superpixel GCN).

Sharding (8 NeuronCores, SPMD): row-shard the HW=65536 pixel dim (8192
rows/core) for the Q^T@x aggregation (pass 1) and Q@H scatter (pass 2).
The GCN itself is column-sharded across cores: core r owns node columns
[r*W,(r+1)*W) of the S=sigmoid(Hx Hx^T) similarity, the masked adjacency
t=S'*A^T, the degree rowsums and the A_hat@V output.  Rank-dependence is
carried entirely by host-sliced per-core inputs (q_own/atp/ics_own) and
by collective layouts (the own-G partial rides the pass-1 AllReduce as a
tail concat; d / H / RP shards are AllGathered), so the compiled program
is rank-independent.  Pass-2 Q^T tiles are prefetched during the GCN.
d^-1/2 uses fused ACT Rsqrt(x+1); leaky-relu runs on DVE as
max(0.01*x, x) to avoid activation-table thrash.
"""

import os
import numpy as np
import ml_dtypes

BF16 = ml_dtypes.bfloat16

HW, C = 65536, 128
NB, NS, NCLS = 1024, 2048, 16
NCORES = 8
EPS = 1e-5
CLAMP = 0.03
FOS = [128, 64, 128, 64]


def _emit(nc, tc, ctx, rows, nb, ns, ncores):
    import concourse.bass as bass
    import concourse.mybir as mybir
    from concourse import masks
    from contextlib import ExitStack

    f32 = mybir.dt.float32
    bf16 = mybir.dt.bfloat16
    ts = bass.ts
    AF = mybir.ActivationFunctionType
    ALU = mybir.AluOpType
    AX = mybir.AxisListType.X

    wb = nb // ncores            # 128
    ws = ns // ncores            # 256

    # ---- dram I/O ----
    din = lambda n_, s, d: nc.dram_tensor(n_, s, d, kind="ExternalInput")
    xs = din("xs", [rows, C], bf16)
    q = din("q", [rows, nb], bf16)
    qs = din("qs", [rows, ns], bf16)
    qbt = din("qbt", [nb, rows], bf16)
    qst = din("qst", [ns, rows], bf16)
    atpb = din("atpb", [nb, wb], bf16)     # A^T[:, own-cols]
    atps = din("atps", [ns, ws], bf16)     # Asmall^T[:, own-cols]
    yte = din("yte", [65, rows], bf16)
    icsb = din("icsb", [1, nb], bf16)
    icss = din("icss", [1, ns], bf16)
    icob = din("icob", [1, wb], bf16)
    icos = din("icos", [1, ws], bf16)
    w128e = din("w128e", [65, 16], bf16)
    wls = [din(f"wl{i}", [128, 256 + 2 * fo + 5], f32) for i, fo in enumerate(FOS)]
    misc = din("misc", [64, 64], f32)
    yo = nc.dram_tensor("yo", [rows, NCLS], f32, kind="ExternalOutput")
    lo = nc.dram_tensor("lo", [rows, NCLS], f32, kind="ExternalOutput")
    dbg = {}
    if os.environ.get("KERNEL_DBG"):
        dbg["g1"] = nc.dram_tensor("dbg_g1", [128, nb + wb], f32,
                                   kind="ExternalOutput")
        dbg["p1"] = nc.dram_tensor("dbg_p1", [128, nb], f32,
                                   kind="ExternalOutput")
        dbg["d_s0"] = nc.dram_tensor("dbg_d_s0", [ns // 128, 128], f32,
                                     kind="ExternalOutput")
        dbg["h1"] = nc.dram_tensor("dbg_h1", [128, nb], f32,
                                   kind="ExternalOutput")
        dbg["rp1"] = nc.dram_tensor("dbg_rp1", [128, (nb // 128) * 32], f32,
                                    kind="ExternalOutput")
        dbg["hto_b"] = nc.dram_tensor("dbg_hto_b", [128, wb], f32,
                                      kind="ExternalOutput")
        dbg["tt_b0"] = nc.dram_tensor("dbg_tt_b0", [128, nb], f32,
                                      kind="ExternalOutput")

    # ---- persistent pools ----
    consts = ctx.enter_context(tc.tile_pool(name="consts", bufs=1))
    gwork = ctx.enter_context(tc.tile_pool(name="gwork", bufs=1))
    dram = ctx.enter_context(tc.tile_pool(name="dram", bufs=1, space="DRAM"))

    ident32 = consts.tile([32, 32], f32)
    masks.make_identity(nc, ident32[:])
    ident16 = consts.tile([16, 16], f32)
    masks.make_identity(nc, ident16[:])
    ones_k1 = consts.tile([1, 128], bf16)
    nc.gpsimd.memset(ones_k1[:], 1.0)
    ones_bf = consts.tile([128, 128], bf16)
    nc.gpsimd.memset(ones_bf[:], 1.0)
    one_col = consts.tile([128, 1], f32)
    nc.gpsimd.memset(one_col[:], 1.0)
    eps_c = consts.tile([128, 1], f32)
    nc.gpsimd.memset(eps_c[:], EPS)

    misc_sb = consts.tile([64, 64], f32)
    nc.sync.dma_start(misc_sb[:], misc[:])
    w128_sb = consts.tile([65, 16], bf16)
    nc.sync.dma_start(w128_sb[:], w128e[:])
    icsb_sb = consts.tile([1, nb], bf16)
    nc.sync.dma_start(icsb_sb[:], icsb[:])
    icss_sb = consts.tile([1, ns], bf16)
    nc.sync.dma_start(icss_sb[:], icss[:])
    icob_sb = consts.tile([1, wb], bf16)
    nc.sync.dma_start(icob_sb[:], icob[:])
    icos_sb = consts.tile([1, ws], bf16)
    nc.sync.dma_start(icos_sb[:], icos[:])
    wl_sb = []
    for i, fo in enumerate(FOS):
        t = consts.tile([128, 256 + 2 * fo + 5], f32, tag=f"wl{i}")
        nc.sync.dma_start(t[:], wls[i][:])
        wl_sb.append(t)
    wcb_bf = consts.tile([64, 32], bf16)
    nc.vector.tensor_copy(wcb_bf[:], misc_sb[:, 0:32])
    wcs_bf = consts.tile([64, 32], bf16)
    nc.vector.tensor_copy(wcs_bf[:], misc_sb[:, 32:64])

    # ---- collective dram tiles ----
    shkw = {"addr_space": "Shared"} if ncores > 4 else {}
    ar1_in = dram.tile([128, nb], f32, tag="ar1i")
    ar1_out = dram.tile([128, nb], f32, tag="ar1o", **shkw)
    ar2_in = dram.tile([128, ns], f32, tag="ar2i")
    ar2_out = dram.tile([128, ns], f32, tag="ar2o", **shkw)
    rs1_in = dram.tile([ncores * 128, wb], f32, tag="rs1i")
    rs1_out = dram.tile([128, wb], f32, tag="rs1o")
    rs2_in = dram.tile([ncores * 128, ws], f32, tag="rs2i")
    rs2_out = dram.tile([128, ws], f32, tag="rs2o")
    agd_bufs = {}
    for br, (n_, w_) in (("b", (nb, wb)), ("s", (ns, ws))):
        for li in range(2):
            agd_bufs[(br, li)] = (
                dram.tile([1, w_], f32, tag=f"agdi{br}{li}",
                          name=f"agdi{br}{li}"),
                dram.tile([n_ // 128, 128], f32, tag=f"agdo{br}{li}",
                          name=f"agdo{br}{li}", **shkw))
    agh_bufs = {
        "b": (dram.tile([128, wb], f32, tag="aghib", name="aghib"),
              dram.tile([ncores * 128, wb], f32, tag="aghob", name="aghob",
                        **shkw)),
        "s": (dram.tile([128, ws], f32, tag="aghis", name="aghis"),
              dram.tile([ncores * 128, ws], f32, tag="aghos", name="aghos",
                        **shkw)),
    }
    agrp_bufs = {
        "b": (dram.tile([wb, 32], f32, tag="agrpib", name="agrpib"),
              dram.tile([nb, 32], f32, tag="agrpob", name="agrpob", **shkw)),
        "s": (dram.tile([ws, 32], f32, tag="agrpis", name="agrpis"),
              dram.tile([ns, 32], f32, tag="agrpos", name="agrpos", **shkw)),
    }
    RG = [list(range(ncores))]

    def allgather(inp, outp):
        nc.gpsimd.collective_compute(
            "AllGather", mybir.AluOpType.bypass, replica_groups=RG,
            ins=[inp.opt()], outs=[outp.opt()])

    # ---- pass 1 ----
    n_rt = rows // 128
    with tc.tile_pool(name="p1pool", bufs=1) as p1pool:
        xall = p1pool.tile([128, n_rt * C], bf16, tag="xall")
        xcnk = n_rt // 4
        for xc in range(4):
            nc.sync.dma_start(
                xall[:, xc * xcnk * C:(xc + 1) * xcnk * C].rearrange(
                    "p (t c) -> p t c", c=C),
                xs[xc * xcnk * 128:(xc + 1) * xcnk * 128, :].rearrange(
                    "(t p) c -> p t c", p=128))

        def pass1_phase(qd, n, g_ps, rgrp, qpool):
            for g in range(n_rt // rgrp):
                r0 = g * rgrp * 128
                r1 = (g + 1) * rgrp * 128
                qt = qpool.tile([128, rgrp * n], bf16, tag="qq")
                nc.gpsimd.dma_start(
                    qt[:].rearrange("p (t c) -> p t c", c=n),
                    qd[r0:r1, :].rearrange("(t p) c -> p t c", p=128))
                for a in range(rgrp):
                    rt = g * rgrp + a
                    xt = xall[:, ts(rt, C)]
                    st = (rt == 0)
                    sp = (rt == n_rt - 1)
                    for cnk in range(n // 512):
                        mv = qt[:, a * n + cnk * 512:a * n + (cnk + 1) * 512]
                        nc.tensor.matmul(g_ps[:, ts(cnk, 512)], xt, mv,
                                         start=st, stop=sp)

        with tc.tile_pool(name="ps_p1b", bufs=1, space="PSUM") as psb, \
             tc.tile_pool(name="qpb", bufs=3) as qpool:
            g1p = psb.tile([128, nb], f32, tag="g1p")
            pass1_phase(q, nb, g1p, 4, qpool)
            gcat1 = p1pool.tile([128, nb], f32, tag="gcat1")
            nc.vector.tensor_copy(gcat1[:], g1p[:])

        if dbg:
            nc.sync.dma_start(dbg["p1"][:], gcat1[:])
        nc.sync.dma_start(ar1_in[:], gcat1[:])
        nc.sync.dma_start(
            rs1_in[:].rearrange("(r f) c -> f r c", f=128),
            gcat1[:].rearrange("f (r c) -> f r c", c=wb))
        nc.gpsimd.collective_compute(
            "AllReduce", mybir.AluOpType.add, replica_groups=RG,
            ins=[ar1_in.opt()], outs=[ar1_out.opt()])
        nc.gpsimd.collective_compute(
            "ReduceScatter", mybir.AluOpType.add, replica_groups=RG,
            ins=[rs1_in.opt()], outs=[rs1_out.opt()])

        with tc.tile_pool(name="ps_p1s", bufs=1, space="PSUM") as pss, \
             tc.tile_pool(name="qps", bufs=3) as qpool:
            g2p = pss.tile([128, ns], f32, tag="g2p")
            pass1_phase(qs, ns, g2p, 2, qpool)
            gcat2 = p1pool.tile([128, ns], f32, tag="gcat2")
            nc.vector.tensor_copy(gcat2[:], g2p[:])

        nc.sync.dma_start(ar2_in[:], gcat2[:])
        nc.sync.dma_start(
            rs2_in[:].rearrange("(r f) c -> f r c", f=128),
            gcat2[:].rearrange("f (r c) -> f r c", c=ws))
        nc.gpsimd.collective_compute(
            "AllReduce", mybir.AluOpType.add, replica_groups=RG,
            ins=[ar2_in.opt()], outs=[ar2_out.opt()])
        nc.gpsimd.collective_compute(
            "ReduceScatter", mybir.AluOpType.add, replica_groups=RG,
            ins=[rs2_in.opt()], outs=[rs2_out.opt()])

    # ---- pass-2 prefetch pool: chunks of 4 j-tiles x GRP pixels, 2 MB ----
    GRP = 2048
    NGRP = rows // GRP
    NJC = 2                     # j-tiles per prefetch chunk
    CPG = (nb + ns) // 128 // NJC   # chunks per group (12)
    PF_BUFS = 8
    pfp = ctx.enter_context(tc.tile_pool(name="pfp", bufs=PF_BUFS))
    pf_tiles = []

    def emit_chunk(qtd, j0, nj, g):
        t = pfp.tile([128, nj * GRP], bf16, tag="pfc")
        nc.gpsimd.dma_start(
            t[:].rearrange("p (j c) -> p j c", c=GRP),
            qtd[j0 * 128:(j0 + nj) * 128,
                g * GRP:(g + 1) * GRP].rearrange("(j p) c -> p j c", p=128))
        pf_tiles.append(t)
        return t

    # consumption order: per group g: big j-chunks then small j-chunks
    chunk_order = []
    for g in range(NGRP):
        for j0 in range(0, nb // 128, NJC):
            chunk_order.append((qbt, j0, NJC, g))
        for j0 in range(0, ns // 128, NJC):
            chunk_order.append((qst, j0, NJC, g))
    for ck in chunk_order[:PF_BUFS]:
        emit_chunk(*ck)
    next_chunk = PF_BUFS

    # ---- GCN: column-sharded, branch-interleaved ----
    # per-branch pools opened up-front in fixed (stack-safe) order; the
    # interleaved generators must not open pools across yields
    gcnctx = ExitStack()
    gcn_pools = {}
    for _brk, _n, _w in (("b", nb, wb), ("s", ns, ws)):
        _bp = gcnctx.enter_context(tc.tile_pool(name=f"b_{_brk}", bufs=1))
        _lp = gcnctx.enter_context(tc.tile_pool(name=f"l_{_brk}", bufs=1))
        _sp = gcnctx.enter_context(tc.tile_pool(name=f"sp_{_brk}", bufs=2))
        _dp = gcnctx.enter_context(
            tc.tile_pool(name=f"psd_{_brk}", bufs=1, space="PSUM"))
        gcn_pools[_brk] = (_bp, _lp, _sp, _dp)

    def gcn_branch(brkey, n, w, ar_out, rs_out, atp_d, ics_sb, ico_sb, lidx,
                   clamp, wc_bf):
        njt = n // 128
        ncnk = n // 512
        packJ = max(512 // w, 1)      # j-tiles per S-pack (big 4, small 2)
        npk = njt // packJ
        if True:
            bp, lp, sp, dpup = gcn_pools[brkey]
            atp_sb = bp.tile([128, njt * w], bf16, tag="atp")
            nc.sync.dma_start(
                atp_sb[:].rearrange("p (j c) -> p j c", c=w),
                atp_d[:].rearrange("(j p) c -> p j c", p=128))

            # L0 input: H = G * (1/colsum), own slice from AR tail
            ht = bp.tile([128, n], f32, tag="hcur0")
            hto = bp.tile([128, w], f32, tag="hto0")
            with tc.tile_pool(name=f"psr_{brkey}", bufs=2, space="PSUM") as psr, \
                 tc.tile_pool(name=f"icsp_{brkey}", bufs=1) as icsp:
                g_sb = icsp.tile([128, n + w], f32, tag="g_sb")
                nc.sync.dma_start(g_sb[:, 0:n], ar_out[:])
                nc.sync.dma_start(g_sb[:, n:n + w], rs_out[:])
                for cnk in range(ncnk):
                    pr = psr.tile([128, 512], f32)
                    nc.tensor.matmul(pr[:], ones_k1[:],
                                     ics_sb[:, ts(cnk, 512)],
                                     start=True, stop=True)
                    nc.vector.tensor_tensor(
                        ht[:, ts(cnk, 512)], g_sb[:, ts(cnk, 512)], pr[:],
                        op=ALU.mult)
                pro = psr.tile([128, w], f32)
                nc.tensor.matmul(pro[:], ones_k1[:], ico_sb[:],
                                 start=True, stop=True)
                nc.vector.tensor_tensor(hto[:], g_sb[:, n:n + w], pro[:],
                                        op=ALU.mult)
                if dbg and brkey == "b":
                    nc.sync.dma_start(dbg["g1"][:], g_sb[:])
                    nc.sync.dma_start(dbg["hto_b"][:], hto[:])

            for li2, wli in enumerate(lidx):
                fo = FOS[wli]
                wl = wl_sb[wli]
                last = (li2 == 1)
                c0 = 256 + fo
                thWT = wl[:, 0:256]
                oWT = wl[:, 256:256 + fo]
                bng = wl[:, c0:c0 + 1]
                bnb = wl[:, c0 + 1:c0 + 2]
                thb = [wl[:, c0 + 2:c0 + 3], wl[:, c0 + 3:c0 + 4]]
                ob_col = wl[0:fo, c0 + 4:c0 + 5]
                ob_rep = wl[:, c0 + 5:c0 + 5 + fo]

                if True:
                    # --- P_bn: batchnorm + hbn + hx (+own) ---
                    s1 = sp.tile([128, 1], f32, tag="s1")
                    nc.vector.reduce_sum(out=s1[:], in_=ht[:], axis=AX)
                    s2p = sp.tile([128, ncnk], f32, tag="s2p")
                    sqs = sp.tile([128, 512], bf16, tag="sqscratch")
                    for cnk in range(ncnk):
                        nc.scalar.activation(
                            sqs[:], ht[:, ts(cnk, 512)], AF.Square,
                            accum_out=s2p[:, cnk:cnk + 1])
                    s2 = sp.tile([128, 1], f32, tag="s2")
                    nc.vector.reduce_sum(out=s2[:], in_=s2p[:], axis=AX)
                    m = sp.tile([128, 1], f32, tag="m")
                    nc.vector.tensor_scalar_mul(m[:], s1[:], 1.0 / n)
                    v = sp.tile([128, 1], f32, tag="v")
                    nc.vector.tensor_scalar_mul(v[:], s2[:], 1.0 / n)
                    m2 = sp.tile([128, 1], f32, tag="m2")
                    nc.vector.tensor_tensor(m2[:], m[:], m[:], op=ALU.mult)
                    nc.vector.tensor_tensor(v[:], v[:], m2[:], op=ALU.subtract)
                    sd = sp.tile([128, 1], f32, tag="sd")
                    nc.scalar.activation(sd[:], v[:], AF.Sqrt, bias=eps_c[:])
                    isd = sp.tile([128, 1], f32, tag="isd")
                    nc.vector.reciprocal(isd[:], sd[:])
                    kk = sp.tile([128, 1], f32, tag="kk")
                    nc.vector.tensor_tensor(kk[:], bng, isd[:], op=ALU.mult)
                    b2 = sp.tile([128, 1], f32, tag="b2")
                    nc.vector.tensor_tensor(b2[:], m[:], kk[:], op=ALU.mult)
                    nc.vector.tensor_tensor(b2[:], bnb, b2[:], op=ALU.subtract)
                    hbn = lp.tile([128, n], bf16, tag="hbn")
                    nc.vector.tensor_scalar(hbn[:], ht[:], kk[:], b2[:],
                                            op0=ALU.mult, op1=ALU.add)
                    hbno = lp.tile([128, w], bf16, tag="hbno")
                    nc.vector.tensor_scalar(hbno[:], hto[:], kk[:], b2[:],
                                            op0=ALU.mult, op1=ALU.add)
                    thWT_bf = lp.tile([128, 256], bf16, tag="thWT_bf")
                    nc.vector.tensor_copy(thWT_bf[:], thWT)
                    oWT_bf = lp.tile([128, fo], bf16, tag=f"oWT_bf{li2}")
                    nc.vector.tensor_copy(oWT_bf[:], oWT)

                    # Hx.T full (lhsT side) + own columns (rhs side)
                    hx = [lp.tile([128, n], bf16, tag=f"hx{k}",
                                  name=f"hx{k}") for k in range(2)]
                    hxo = [lp.tile([128, w], bf16, tag=f"hxo{k}",
                                   name=f"hxo{k}") for k in range(2)]
                    with tc.tile_pool(name=f"psx_{brkey}{li2}", bufs=2,
                                      space="PSUM") as psx:
                        for k in range(2):
                            for cnk in range(ncnk):
                                px = psx.tile([128, 512], f32)
                                nc.tensor.matmul(
                                    px[:], thWT_bf[:, ts(k, 128)],
                                    hbn[:, ts(cnk, 512)],
                                    start=True, stop=True)
                                nc.vector.tensor_scalar_add(
                                    hx[k][:, ts(cnk, 512)], px[:], thb[k])
                            pxo = psx.tile([128, w], f32)
                            nc.tensor.matmul(pxo[:], thWT_bf[:, ts(k, 128)],
                                             hbno[:], start=True, stop=True)
                            nc.vector.tensor_scalar_add(hxo[k][:], pxo[:],
                                                        thb[k])
                    yield

                    # --- P_s: S packs -> sigmoid -> t ; d rowsums; V; pus ---
                    ttp = lp.tile([128, njt * w], bf16, tag="ttp")
                    dpu = dpup.tile([128, 2 * w], f32, tag="dpu")
                    dpsum = dpu[:, 0:w]
                    pus = dpu[0:fo, w:2 * w]
                    with tc.tile_pool(name=f"pss_{brkey}{li2}", bufs=2,
                                      space="PSUM") as pssb:
                        for pk in range(npk):
                            px = pssb.tile([128, packJ * w], f32, tag="spack")
                            for jj in range(packJ):
                                j = pk * packJ + jj
                                nc.tensor.matmul(px[:, ts(jj, w)],
                                                 hx[0][:, ts(j, 128)],
                                                 hxo[0][:],
                                                 start=True, stop=False)
                                nc.tensor.matmul(px[:, ts(jj, w)],
                                                 hx[1][:, ts(j, 128)],
                                                 hxo[1][:],
                                                 start=False, stop=True)
                            sbl = sp.tile([128, packJ * w], bf16, tag="sblk")
                            nc.scalar.activation(sbl[:], px[:], AF.Sigmoid)
                            tsl = ttp[:, pk * packJ * w:(pk + 1) * packJ * w]
                            asl = atp_sb[:, pk * packJ * w:(pk + 1) * packJ * w]
                            if clamp:
                                nc.vector.scalar_tensor_tensor(
                                    tsl, sbl[:], CLAMP, asl,
                                    op0=ALU.max, op1=ALU.mult)
                            else:
                                nc.vector.tensor_tensor(tsl, sbl[:], asl,
                                                        op=ALU.mult)

                    for j in range(njt):
                        nc.tensor.matmul(dpsum, ones_bf[:],
                                         ttp[:, ts(j, w)],
                                         start=(j == 0), stop=(j == njt - 1))
                    if dbg and brkey == "b" and li2 == 0:
                        ttf = lp.tile([128, njt * w], f32, tag="ttf")
                        nc.vector.tensor_copy(ttf[:], ttp[:])
                        nc.sync.dma_start(dbg["tt_b0"][:], ttf[:])
                    drow = lp.tile([1, w], f32, tag="drow")
                    nc.vector.tensor_copy(drow[:], dpu[0:1, 0:w])
                    agd_in, agd_out = agd_bufs[(brkey, li2)]
                    nc.sync.dma_start(agd_in[:], drow[:])
                    allgather(agd_in, agd_out)

                    # V_j = H@oW + ob for all j (stats base), pus for own cols
                    stats = lp.tile([128, njt * fo], bf16, tag=f"stats{li2}")
                    nc.tensor.matmul(pus, oWT_bf[:], hbno[:],
                                     start=True, stop=True)
                    with tc.tile_pool(name=f"psh_{brkey}{li2}", bufs=1,
                                      space="PSUM") as psh:
                        for j in range(njt):
                            ph = psh.tile([128, fo], f32)
                            nc.tensor.matmul(ph[:], hbn[:, ts(j, 128)],
                                             oWT_bf[:], start=True, stop=True)
                            nc.vector.tensor_tensor(stats[:, ts(j, fo)],
                                                    ph[:], ob_rep, op=ALU.add)
                    yield

                    # --- P_d: rsqrt(d), scale, out matmul, epilogue ---
                    prcs = lp.tile([128, w], f32, tag="prcs")
                    nc.scalar.activation(prcs[:], dpsum, AF.Sqrt,
                                         bias=one_col[:])
                    prc = lp.tile([128, w], f32, tag="prc")
                    nc.vector.reciprocal(prc[:], prcs[:])
                    dcol = lp.tile([128, njt], f32, tag="dcol")
                    rsqs = lp.tile([njt, 128], f32, tag="rsqs")
                    rsqn = lp.tile([njt, 128], f32, tag="rsqn")
                    rsqi = lp.tile([njt, 128], f32, tag="rsqi")
                    nc.sync.dma_start(rsqi[:], agd_out[:])
                    if dbg and brkey == "s" and li2 == 0:
                        nc.sync.dma_start(dbg["d_s0"][:], rsqi[:])
                    nc.scalar.activation(rsqs[:], rsqi[:], AF.Sqrt,
                                         bias=one_col[0:njt, :])
                    nc.vector.reciprocal(rsqn[:], rsqs[:])
                    with tc.tile_pool(name=f"pst_{brkey}{li2}", bufs=1,
                                      space="PSUM") as pst:
                        pt = pst.tile([128, njt], f32, tag="dt")
                        nc.tensor.transpose(pt[:], rsqn[:],
                                            ident16[0:njt, 0:njt])
                        nc.vector.tensor_copy(dcol[:], pt[:])
                    for j in range(njt):
                        nc.vector.tensor_scalar_mul(stats[:, ts(j, fo)],
                                                    stats[:, ts(j, fo)],
                                                    dcol[:, j:j + 1])
                    z1 = lp.tile([fo, w], f32, tag=f"z1{li2}")
                    nc.vector.scalar_tensor_tensor(
                        z1[:], pus, ob_col, prc[0:fo, :],
                        op0=ALU.add, op1=ALU.mult)
                    with tc.tile_pool(name=f"pso_{brkey}{li2}", bufs=1,
                                      space="PSUM") as pso:
                        po = pso.tile([fo, w], f32, tag="po")
                        for j in range(njt):
                            nc.tensor.matmul(po[:], stats[:, ts(j, fo)],
                                             ttp[:, ts(j, w)],
                                             start=(j == 0),
                                             stop=(j == njt - 1))
                        s2t = lp.tile([fo, w], f32, tag=f"s2t{li2}")
                        nc.vector.tensor_tensor(s2t[:], po[:], z1[:],
                                                op=ALU.add)
                    s3t = lp.tile([fo, w], f32, tag=f"s3t{li2}")
                    nc.vector.tensor_tensor(s3t[:], s2t[:], prc[0:fo, :],
                                            op=ALU.mult)
                    if not last:
                        hnext = bp.tile([128, w], f32, tag="hto1")
                        nc.vector.scalar_tensor_tensor(
                            hnext[0:fo, :], s3t[:], 0.01, s3t[:],
                            op0=ALU.mult, op1=ALU.max)
                        agh_in, agh_out = agh_bufs[brkey]
                        nc.sync.dma_start(agh_in[0:fo, :], hnext[0:fo, :])
                        allgather(agh_in, agh_out)
                    else:
                        hnb = lp.tile([fo, w], bf16, tag="hnb")
                        nc.vector.scalar_tensor_tensor(
                            hnb[:], s3t[:], 0.01, s3t[:],
                            op0=ALU.mult, op1=ALU.max)
                        agrp_in, agrp_out = agrp_bufs[brkey]
                        with tc.tile_pool(name=f"psq_{brkey}", bufs=1,
                                          space="PSUM") as psq:
                            for i in range(w // 128):
                                pr = psq.tile([128, 32], f32)
                                nc.tensor.matmul(pr[:], hnb[:, ts(i, 128)],
                                                 wc_bf[:], start=True,
                                                 stop=True)
                                rpo = lp.tile([128, 32], f32, tag="rpo")
                                nc.vector.tensor_copy(rpo[:], pr[:])
                                nc.sync.dma_start(
                                    agrp_in[i * 128:(i + 1) * 128, :], rpo[:])
                        allgather(agrp_in, agrp_out)
                yield

                # --- P_re: reload full H for next layer ---
                if not last:
                    agh_in, agh_out = agh_bufs[brkey]
                    htn = bp.tile([fo, n], f32, tag=f"hfull{li2}")
                    nc.sync.dma_start(
                        htn[:].rearrange("f (r c) -> f r c", c=w),
                        agh_out[:].rearrange("(r f) c -> f r c", f=128))
                    if dbg and brkey == "b":
                        nc.sync.dma_start(dbg["h1"][:], htn[:])
                    ht = htn
                    hto = hnext

    gb = gcn_branch("b", nb, wb, ar1_out, rs1_out, atpb, icsb_sb, icob_sb,
                    [0, 1], True, wcb_bf)
    gs = gcn_branch("s", ns, ws, ar2_out, rs2_out, atps, icss_sb, icos_sb,
                    [2, 3], False, wcs_bf)
    # drive: big L0 fully first (AR1 ready before AR2), then alternate
    for _ in range(3):
        next(gb)
    gb_done = gs_done = False
    while not (gb_done and gs_done):
        if not gs_done:
            try:
                next(gs)
            except StopIteration:
                gs_done = True
        if not gb_done:
            try:
                next(gb)
            except StopIteration:
                gb_done = True
    gcnctx.close()

    # ---- rp stationaries from AllGathered node shards ----
    rp1 = gwork.tile([128, (nb // 128) * 32], bf16, tag="rp1")
    rp1f = gwork.tile([128, (nb // 128) * 32], f32, tag="rp1f")
    nc.sync.dma_start(
        rp1f[:].rearrange("p (j c) -> p j c", c=32),
        agrp_bufs["b"][1][:].rearrange("(j p) c -> p j c", p=128))
    nc.vector.tensor_copy(rp1[:], rp1f[:])
    if dbg:
        nc.sync.dma_start(dbg["rp1"][:], rp1f[:])
    rp2 = gwork.tile([128, (ns // 128) * 32], bf16, tag="rp2")
    rp2f = gwork.tile([128, (ns // 128) * 32], f32, tag="rp2f")
    nc.sync.dma_start(
        rp2f[:].rearrange("p (j c) -> p j c", c=32),
        agrp_bufs["s"][1][:].rearrange("(j p) c -> p j c", p=128))
    nc.vector.tensor_copy(rp2[:], rp2f[:])

    # ---- pass 2 + epilogue ----
    nrc = GRP // 512
    CH = 512
    with tc.tile_pool(name="ytp", bufs=2) as ytp, \
         tc.tile_pool(name="ps_z", bufs=1, space="PSUM") as ps_z, \
         tc.tile_pool(name="ps_yw", bufs=2, space="PSUM") as ps_yw, \
         tc.tile_pool(name="ps_tp", bufs=2, space="PSUM") as ps_tp, \
         tc.tile_pool(name="epil", bufs=4) as ep:
        for gidx in range(NGRP):
            ytt = ytp.tile([65, GRP], bf16, tag="ytt")
            nc.sync.dma_start(ytt[:], yte[:, gidx * GRP:(gidx + 1) * GRP])
            pz = ps_z.tile([32, GRP], f32, tag="pz", name=f"pz{gidx}")
            ci = gidx * CPG
            nbc = nb // 128 // NJC

            def after_consume(c):
                # chunk c's consumers are emitted; its slot reuser (c+PF_BUFS)
                # may now be emitted without clobbering un-consumed data
                nonlocal next_chunk
                while next_chunk < len(chunk_order) and \
                        next_chunk <= c + PF_BUFS:
                    emit_chunk(*chunk_order[next_chunk])
                    next_chunk += 1

            for j in range(nb // 128):
                tq = pf_tiles[ci + j // NJC][:, ts(j % NJC, GRP)]
                for rc in range(nrc):
                    nc.tensor.matmul(pz[:, ts(rc, CH)], rp1[:, ts(j, 32)],
                                     tq[:, ts(rc, CH)],
                                     start=(j == 0), stop=False)
                if (j + 1) % NJC == 0:
                    after_consume(ci + j // NJC)
            for j in range(ns // 128):
                tq = pf_tiles[ci + nbc + j // NJC][:, ts(j % NJC, GRP)]
                for rc in range(nrc):
                    nc.tensor.matmul(pz[:, ts(rc, CH)], rp2[:, ts(j, 32)],
                                     tq[:, ts(rc, CH)],
                                     start=False, stop=(j == ns // 128 - 1))
                if (j + 1) % NJC == 0:
                    after_consume(ci + nbc + j // NJC)
            for rc in range(nrc):
                base = gidx * GRP + rc * CH
                pyw = ps_yw.tile([16, CH], f32)
                nc.tensor.matmul(pyw[:], w128_sb[:], ytt[:, ts(rc, CH)],
                                 start=True, stop=True)
                yws = ep.tile([16, CH], f32, tag="yws")
                nc.scalar.activation(yws[:], pyw[:], AF.Copy)
                tri = ep.tile([32, CH], f32, tag="tri")
                nc.scalar.activation(tri[:], pz[:, ts(rc, CH)], AF.Copy)
                nc.vector.tensor_tensor(tri[0:16, :], pz[0:16, ts(rc, CH)],
                                        yws[:], op=ALU.add)
                for s in range(CH // 128):
                    ptr = ps_tp.tile([128, 32], f32)
                    nc.tensor.transpose(ptr[:], tri[:, ts(s, 128)], ident32[:])
                    e = ep.tile([128, 16], f32, tag="e")
                    ssum = ep.tile([128, 1], f32, tag="ssum")
                    nc.scalar.activation(e[:], ptr[:, 0:16], AF.Exp,
                                         accum_out=ssum[:])
                    rcp = ep.tile([128, 1], f32, tag="rcp")
                    nc.vector.reciprocal(rcp[:], ssum[:])
                    yot = ep.tile([128, 16], f32, tag="yot")
                    nc.vector.tensor_scalar_mul(yot[:], e[:], rcp[:])
                    lot = ep.tile([128, 16], f32, tag="lot")
                    nc.scalar.activation(lot[:], ptr[:, 16:32], AF.Square)
                    nc.sync.dma_start(
                        yo[base + s * 128:base + (s + 1) * 128, :], yot[:])
                    nc.sync.dma_start(
                        lo[base + s * 128:base + (s + 1) * 128, :], lot[:])


def build(rows=HW // NCORES, nb=NB, ns=NS, ncores=NCORES):
    from contextlib import ExitStack
    import concourse.bacc as bacc
    import concourse.tile as tile

    nc = bacc.Bacc("TRN2", target_bir_lowering=False, debug=False,
                   enable_asserts=True, num_devices=ncores)
    with tile.TileContext(nc) as tc:
        with ExitStack() as ctx:
            _emit(nc, tc, ctx, rows, nb, ns, ncores)
    nc.compile()
    return nc


# --------------------------------------------------------------------------
# host wrapper
# --------------------------------------------------------------------------

def prep_inputs(rows, nb, ns, ncores,
                x, y, Q, A, Qsmall, Asmall,
                b0_bng, b0_bnb, b0_thW, b0_thb, b0_oW, b0_ob,
                b1_bng, b1_bnb, b1_thW, b1_thb, b1_oW, b1_ob,
                s0_bng, s0_bnb, s0_thW, s0_thb, s0_oW, s0_ob,
                s1_bng, s1_bnb, s1_thW, s1_thb, s1_oW, s1_ob,
                lin128_W, lin128_b, lin64_W, lin64_b, sigma2):
    f = np.float32
    hw = rows * ncores
    wb, ws = nb // ncores, ns // ncores
    flat = np.ascontiguousarray(np.asarray(x, f).reshape(hw, -1))
    Q = np.asarray(Q, f)
    Qs = np.asarray(Qsmall, f)
    y = np.asarray(y, f)

    icsb = (1.0 / Q.astype(BF16).astype(f).sum(axis=0))[None, :].astype(BF16)
    icss = (1.0 / Qs.astype(BF16).astype(f).sum(axis=0))[None, :].astype(BF16)

    def wl_pack(thW, thb, oW, ob, bng, bnb):
        fo = np.asarray(oW).shape[0]
        w = np.zeros((128, 256 + 2 * fo + 5), f)
        w[:, 0:256] = np.asarray(thW, f).T
        w[:, 256:256 + fo] = np.asarray(oW, f).T
        c0 = 256 + fo
        w[:, c0] = np.asarray(bng, f)
        w[:, c0 + 1] = np.asarray(bnb, f)
        w[:, c0 + 2] = np.asarray(thb, f)[0:128]
        w[:, c0 + 3] = np.asarray(thb, f)[128:256]
        w[0:fo, c0 + 4] = np.asarray(ob, f)
        w[:, c0 + 5:c0 + 5 + fo] = np.asarray(ob, f)[None, :]
        return w

    wl = [
        wl_pack(b0_thW, b0_thb, b0_oW, b0_ob, b0_bng, b0_bnb),
        wl_pack(b1_thW, b1_thb, b1_oW, b1_ob, b1_bng, b1_bnb),
        wl_pack(s0_thW, s0_thb, s0_oW, s0_ob, s0_bng, s0_bnb),
        wl_pack(s1_thW, s1_thb, s1_oW, s1_ob, s1_bng, s1_bnb),
    ]

    sig = float(np.asarray(sigma2).reshape(-1)[0])
    W128 = np.asarray(lin128_W, f)
    W64 = np.asarray(lin64_W, f)
    misc = np.zeros((64, 64), f)
    misc[:, 0:16] = sig * W128[:, :64].T
    misc[:, 16:32] = W64.T
    misc[:, 32:48] = (1.0 - sig) * W128[:, :64].T
    misc[:, 48:64] = -W64.T

    w128e = np.zeros((65, 16), f)
    w128e[0:64, :] = W128[:, 64:].T
    w128e[64, :] = np.asarray(lin128_b, f)
    w128e = w128e.astype(BF16)

    at_b = np.ascontiguousarray(np.asarray(A, f).T).astype(BF16)
    ast_b = np.ascontiguousarray(np.asarray(Asmall, f).T).astype(BF16)

    in_maps = []
    for c in range(ncores):
        r0, r1 = c * rows, (c + 1) * rows
        qsh = Q[r0:r1]
        qssh = Qs[r0:r1]
        yte = np.ones((65, rows), f)
        yte[0:64, :] = y[r0:r1].T
        m = {
            "xs": flat[r0:r1].astype(BF16),
            "q": qsh.astype(BF16),
            "qs": qssh.astype(BF16),
            "qbt": np.ascontiguousarray(qsh.T).astype(BF16),
            "qst": np.ascontiguousarray(qssh.T).astype(BF16),
            "atpb": np.ascontiguousarray(at_b[:, c * wb:(c + 1) * wb]),
            "atps": np.ascontiguousarray(ast_b[:, c * ws:(c + 1) * ws]),
            "yte": yte.astype(BF16),
            "icsb": icsb,
            "icss": icss,
            "icob": np.ascontiguousarray(icsb[:, c * wb:(c + 1) * wb]),
            "icos": np.ascontiguousarray(icss[:, c * ws:(c + 1) * ws]),
            "w128e": w128e,
            "misc": misc,
        }
        for i in range(4):
            m[f"wl{i}"] = wl[i]
        in_maps.append(m)
    return in_maps


_cache = {}
_last_results = None


def _ensure_ntff_hook():
    """Register the axon NTFF profile hook if the image's antenv lacks it."""
    import sys, types, ctypes, contextlib
    try:
        from antenv.axon_hooks import get_axon_ntff_profile_hook  # noqa: F401
        return True
    except ImportError:
        pass
    so_path = "/opt/axon/libaxon_pjrt.so"
    if not os.path.exists(so_path):
        return False
    lib = ctypes.CDLL(so_path)
    if not hasattr(lib, "axon_start_nrt_profile"):
        return False
    lib.axon_start_nrt_profile.argtypes = [ctypes.POINTER(ctypes.c_int64),
                                           ctypes.c_size_t]
    lib.axon_start_nrt_profile.restype = ctypes.c_int64
    lib.axon_stop_nrt_profile.argtypes = [ctypes.c_char_p]
    lib.axon_stop_nrt_profile.restype = ctypes.c_int64

    @contextlib.contextmanager
    def _hook(output_dir, device_ids):
        import jax
        jax.devices()
        if device_ids:
            ids = (ctypes.c_int64 * len(device_ids))(*device_ids)
            rc = lib.axon_start_nrt_profile(ids, len(device_ids))
        else:
            rc = lib.axon_start_nrt_profile(None, 0)
        if rc != 0:
            raise RuntimeError(f"axon_start_nrt_profile rc={rc}")
        try:
            yield
        finally:
            n = lib.axon_stop_nrt_profile(str(output_dir).encode())
            print(f"profile: {n} file(s) written to {output_dir}",
                  file=sys.stderr)

    mod = types.ModuleType("antenv.axon_hooks")
    holder = [_hook]
    mod.get_axon_ntff_profile_hook = lambda: holder[0]
    mod.set_axon_ntff_profile_hook = lambda h: holder.__setitem__(0, h)
    sys.modules["antenv.axon_hooks"] = mod
    import antenv
    antenv.axon_hooks = mod
    return True


def kernel(**inputs):
    global _last_results
    if "nc" not in _cache:
        _cache["nc"] = build()
    nc = _cache["nc"]
    rows = HW // NCORES
    in_maps = prep_inputs(rows, NB, NS, NCORES, **inputs)
    from concourse.bass_utils import run_bass_kernel_spmd
    trace = bool(os.environ.get("KERNEL_TRACE")) and _ensure_ntff_hook()
    res = run_bass_kernel_spmd(nc, in_maps, core_ids=list(range(NCORES)),
                               trace=trace)
    _last_results = res
    Y = np.concatenate([np.asarray(r["yo"]) for r in res.results], axis=0)
    L = np.concatenate([np.asarray(r["lo"]) for r in res.results], axis=0)
    return Y, L


# revision 24
# speedup vs baseline: 1.2262x; 1.0305x over previous
"""Trainium2 Bass kernel for nn_DSR_GCN (dual-superpixel GCN).

Sharding (8 NeuronCores, SPMD): row-shard the HW=65536 pixel dim (8192
rows/core) for the Q^T@x aggregation (pass 1) and Q@H scatter (pass 2).
The GCN itself is column-sharded across cores: core r owns node columns
[r*W,(r+1)*W) of the S=sigmoid(Hx Hx^T) similarity, the masked adjacency
t=S'*A^T, the degree rowsums and the A_hat@V output.  Rank-dependence is
carried entirely by host-sliced per-core inputs (q_own/atp/ics_own) and
by collective layouts (the own-G partial rides the pass-1 AllReduce as a
tail concat; d / H / RP shards are AllGathered), so the compiled program
is rank-independent.  Pass-2 Q^T tiles are prefetched during the GCN.
d^-1/2 uses fused ACT Rsqrt(x+1); leaky-relu runs on DVE as
max(0.01*x, x) to avoid activation-table thrash.
"""

import os
import numpy as np
import ml_dtypes

BF16 = ml_dtypes.bfloat16

HW, C = 65536, 128
NB, NS, NCLS = 1024, 2048, 16
NCORES = 8
EPS = 1e-5
CLAMP = 0.03
FOS = [128, 64, 128, 64]


def _emit(nc, tc, ctx, rows, nb, ns, ncores):
    import concourse.bass as bass
    import concourse.mybir as mybir
    from concourse import masks
    from contextlib import ExitStack

    f32 = mybir.dt.float32
    bf16 = mybir.dt.bfloat16
    ts = bass.ts
    AF = mybir.ActivationFunctionType
    ALU = mybir.AluOpType
    AX = mybir.AxisListType.X

    wb = nb // ncores            # 128
    ws = ns // ncores            # 256

    # ---- dram I/O ----
    din = lambda n_, s, d: nc.dram_tensor(n_, s, d, kind="ExternalInput")
    xs = din("xs", [rows, C], bf16)
    q = din("q", [rows, nb], bf16)
    qs = din("qs", [rows, ns], bf16)
    qbt = din("qbt", [nb, rows], bf16)
    qst = din("qst", [ns, rows], bf16)
    atpb = din("atpb", [nb, wb], bf16)     # A^T[:, own-cols]
    atps = din("atps", [ns, ws], bf16)     # Asmall^T[:, own-cols]
    yte = din("yte", [65, rows], bf16)
    icsb = din("icsb", [1, nb], bf16)
    icss = din("icss", [1, ns], bf16)
    icob = din("icob", [1, wb], bf16)
    icos = din("icos", [1, ws], bf16)
    w128e = din("w128e", [65, 16], bf16)
    wls = [din(f"wl{i}", [128, 256 + 2 * fo + 5], f32) for i, fo in enumerate(FOS)]
    misc = din("misc", [64, 64], f32)
    yo = nc.dram_tensor("yo", [rows, NCLS], f32, kind="ExternalOutput")
    lo = nc.dram_tensor("lo", [rows, NCLS], f32, kind="ExternalOutput")
    dbg = {}
    if os.environ.get("KERNEL_DBG"):
        dbg["g1"] = nc.dram_tensor("dbg_g1", [128, nb + wb], f32,
                                   kind="ExternalOutput")
        dbg["p1"] = nc.dram_tensor("dbg_p1", [128, nb], f32,
                                   kind="ExternalOutput")
        dbg["d_s0"] = nc.dram_tensor("dbg_d_s0", [ns // 128, 128], f32,
                                     kind="ExternalOutput")
        dbg["h1"] = nc.dram_tensor("dbg_h1", [128, nb], f32,
                                   kind="ExternalOutput")
        dbg["rp1"] = nc.dram_tensor("dbg_rp1", [128, (nb // 128) * 32], f32,
                                    kind="ExternalOutput")
        dbg["hto_b"] = nc.dram_tensor("dbg_hto_b", [128, wb], f32,
                                      kind="ExternalOutput")
        dbg["tt_b0"] = nc.dram_tensor("dbg_tt_b0", [128, nb], f32,
                                      kind="ExternalOutput")

    # ---- persistent pools ----
    consts = ctx.enter_context(tc.tile_pool(name="consts", bufs=1))
    gwork = ctx.enter_context(tc.tile_pool(name="gwork", bufs=1))
    dram = ctx.enter_context(tc.tile_pool(name="dram", bufs=1, space="DRAM"))

    ident32 = consts.tile([32, 32], f32)
    masks.make_identity(nc, ident32[:])
    ident16 = consts.tile([16, 16], f32)
    masks.make_identity(nc, ident16[:])
    ones_k1 = consts.tile([1, 128], bf16)
    nc.gpsimd.memset(ones_k1[:], 1.0)
    ones_bf = consts.tile([128, 128], bf16)
    nc.gpsimd.memset(ones_bf[:], 1.0)
    one_col = consts.tile([128, 1], f32)
    nc.gpsimd.memset(one_col[:], 1.0)
    eps_c = consts.tile([128, 1], f32)
    nc.gpsimd.memset(eps_c[:], EPS)

    misc_sb = consts.tile([64, 64], f32)
    nc.sync.dma_start(misc_sb[:], misc[:])
    w128_sb = consts.tile([65, 16], bf16)
    nc.sync.dma_start(w128_sb[:], w128e[:])
    icsb_sb = consts.tile([1, nb], bf16)
    nc.sync.dma_start(icsb_sb[:], icsb[:])
    icss_sb = consts.tile([1, ns], bf16)
    nc.sync.dma_start(icss_sb[:], icss[:])
    icob_sb = consts.tile([1, wb], bf16)
    nc.sync.dma_start(icob_sb[:], icob[:])
    icos_sb = consts.tile([1, ws], bf16)
    nc.sync.dma_start(icos_sb[:], icos[:])
    wl_sb = []
    for i, fo in enumerate(FOS):
        t = consts.tile([128, 256 + 2 * fo + 5], f32, tag=f"wl{i}")
        nc.sync.dma_start(t[:], wls[i][:])
        wl_sb.append(t)
    wcb_bf = consts.tile([64, 32], bf16)
    nc.vector.tensor_copy(wcb_bf[:], misc_sb[:, 0:32])
    wcs_bf = consts.tile([64, 32], bf16)
    nc.vector.tensor_copy(wcs_bf[:], misc_sb[:, 32:64])

    # ---- collective dram tiles ----
    shkw = {"addr_space": "Shared"} if ncores > 4 else {}
    ar1_in = dram.tile([128, nb], f32, tag="ar1i")
    ar1_out = dram.tile([128, nb], f32, tag="ar1o", **shkw)
    ar2_in = dram.tile([128, ns], f32, tag="ar2i")
    ar2_out = dram.tile([128, ns], f32, tag="ar2o", **shkw)
    rs1_in = dram.tile([ncores * 128, wb], f32, tag="rs1i")
    rs1_out = dram.tile([128, wb], f32, tag="rs1o")
    rs2_in = dram.tile([ncores * 128, ws], f32, tag="rs2i")
    rs2_out = dram.tile([128, ws], f32, tag="rs2o")
    agd_bufs = {}
    for br, (n_, w_) in (("b", (nb, wb)), ("s", (ns, ws))):
        for li in range(2):
            agd_bufs[(br, li)] = (
                dram.tile([1, w_], f32, tag=f"agdi{br}{li}",
                          name=f"agdi{br}{li}"),
                dram.tile([n_ // 128, 128], f32, tag=f"agdo{br}{li}",
                          name=f"agdo{br}{li}", **shkw))
    agh_bufs = {
        "b": (dram.tile([128, wb], f32, tag="aghib", name="aghib"),
              dram.tile([ncores * 128, wb], f32, tag="aghob", name="aghob",
                        **shkw)),
        "s": (dram.tile([128, ws], f32, tag="aghis", name="aghis"),
              dram.tile([ncores * 128, ws], f32, tag="aghos", name="aghos",
                        **shkw)),
    }
    agrp_bufs = {
        "b": (dram.tile([wb, 32], f32, tag="agrpib", name="agrpib"),
              dram.tile([nb, 32], f32, tag="agrpob", name="agrpob", **shkw)),
        "s": (dram.tile([ws, 32], f32, tag="agrpis", name="agrpis"),
              dram.tile([ns, 32], f32, tag="agrpos", name="agrpos", **shkw)),
    }
    RG = [list(range(ncores))]

    def allgather(inp, outp):
        nc.gpsimd.collective_compute(
            "AllGather", mybir.AluOpType.bypass, replica_groups=RG,
            ins=[inp.opt()], outs=[outp.opt()])

    # ---- pass 1 ----
    n_rt = rows // 128
    with tc.tile_pool(name="p1pool", bufs=1) as p1pool:
        xall = p1pool.tile([128, n_rt * C], bf16, tag="xall")
        xcnk = n_rt // 4
        for xc in range(4):
            nc.sync.dma_start(
                xall[:, xc * xcnk * C:(xc + 1) * xcnk * C].rearrange(
                    "p (t c) -> p t c", c=C),
                xs[xc * xcnk * 128:(xc + 1) * xcnk * 128, :].rearrange(
                    "(t p) c -> p t c", p=128))

        def pass1_phase(qd, n, g_ps, rgrp, qpool):
            for g in range(n_rt // rgrp):
                r0 = g * rgrp * 128
                r1 = (g + 1) * rgrp * 128
                qt = qpool.tile([128, rgrp * n], bf16, tag="qq")
                nc.gpsimd.dma_start(
                    qt[:].rearrange("p (t c) -> p t c", c=n),
                    qd[r0:r1, :].rearrange("(t p) c -> p t c", p=128))
                for a in range(rgrp):
                    rt = g * rgrp + a
                    xt = xall[:, ts(rt, C)]
                    st = (rt == 0)
                    sp = (rt == n_rt - 1)
                    for cnk in range(n // 512):
                        mv = qt[:, a * n + cnk * 512:a * n + (cnk + 1) * 512]
                        nc.tensor.matmul(g_ps[:, ts(cnk, 512)], xt, mv,
                                         start=st, stop=sp)

        with tc.tile_pool(name="ps_p1b", bufs=1, space="PSUM") as psb, \
             tc.tile_pool(name="qpb", bufs=3) as qpool:
            g1p = psb.tile([128, nb], f32, tag="g1p")
            pass1_phase(q, nb, g1p, 4, qpool)
            gcat1 = p1pool.tile([128, nb], f32, tag="gcat1")
            nc.vector.tensor_copy(gcat1[:], g1p[:])

        if dbg:
            nc.sync.dma_start(dbg["p1"][:], gcat1[:])
        nc.sync.dma_start(ar1_in[:], gcat1[:])
        nc.sync.dma_start(
            rs1_in[:].rearrange("(r f) c -> f r c", f=128),
            gcat1[:].rearrange("f (r c) -> f r c", c=wb))
        nc.gpsimd.collective_compute(
            "AllReduce", mybir.AluOpType.add, replica_groups=RG,
            ins=[ar1_in.opt()], outs=[ar1_out.opt()])
        nc.gpsimd.collective_compute(
            "ReduceScatter", mybir.AluOpType.add, replica_groups=RG,
            ins=[rs1_in.opt()], outs=[rs1_out.opt()])

        with tc.tile_pool(name="ps_p1s", bufs=1, space="PSUM") as pss, \
             tc.tile_pool(name="qps", bufs=3) as qpool:
            g2p = pss.tile([128, ns], f32, tag="g2p")
            pass1_phase(qs, ns, g2p, 2, qpool)
            gcat2 = p1pool.tile([128, ns], f32, tag="gcat2")
            nc.vector.tensor_copy(gcat2[:], g2p[:])

        nc.sync.dma_start(ar2_in[:], gcat2[:])
        nc.sync.dma_start(
            rs2_in[:].rearrange("(r f) c -> f r c", f=128),
            gcat2[:].rearrange("f (r c) -> f r c", c=ws))

    def trigger_ar2():
        # deferred: emitted after GCN-big L0's AG triggers so the big branch
        # fills the AR2/RS2 wait instead of stalling the gpsimd queue
        nc.gpsimd.collective_compute(
            "AllReduce", mybir.AluOpType.add, replica_groups=RG,
            ins=[ar2_in.opt()], outs=[ar2_out.opt()])
        nc.gpsimd.collective_compute(
            "ReduceScatter", mybir.AluOpType.add, replica_groups=RG,
            ins=[rs2_in.opt()], outs=[rs2_out.opt()])

    # ---- pass-2 prefetch pool: chunks of 4 j-tiles x GRP pixels, 2 MB ----
    GRP = 2048
    NGRP = rows // GRP
    NJC = 2                     # j-tiles per prefetch chunk
    CPG = (nb + ns) // 128 // NJC   # chunks per group (12)
    PF_BUFS = 9
    pfp = ctx.enter_context(tc.tile_pool(name="pfp", bufs=PF_BUFS))
    pf_tiles = []

    def emit_chunk(qtd, j0, nj, g):
        t = pfp.tile([128, nj * GRP], bf16, tag="pfc")
        nc.gpsimd.dma_start(
            t[:].rearrange("p (j c) -> p j c", c=GRP),
            qtd[j0 * 128:(j0 + nj) * 128,
                g * GRP:(g + 1) * GRP].rearrange("(j p) c -> p j c", p=128))
        pf_tiles.append(t)
        return t

    # consumption order: per group g: big j-chunks then small j-chunks
    chunk_order = []
    for g in range(NGRP):
        for j0 in range(0, nb // 128, NJC):
            chunk_order.append((qbt, j0, NJC, g))
        for j0 in range(0, ns // 128, NJC):
            chunk_order.append((qst, j0, NJC, g))
    for ck in chunk_order[:PF_BUFS]:
        emit_chunk(*ck)
    next_chunk = PF_BUFS

    # ---- GCN: column-sharded, branch-interleaved ----
    # per-branch pools opened up-front in fixed (stack-safe) order; the
    # interleaved generators must not open pools across yields
    gcnctx = ExitStack()
    gcn_pools = {}
    for _brk, _n, _w in (("b", nb, wb), ("s", ns, ws)):
        _bp = gcnctx.enter_context(tc.tile_pool(name=f"b_{_brk}", bufs=1))
        _lp = gcnctx.enter_context(tc.tile_pool(name=f"l_{_brk}", bufs=1))
        _sp = gcnctx.enter_context(tc.tile_pool(name=f"sp_{_brk}", bufs=2))
        _dp = gcnctx.enter_context(
            tc.tile_pool(name=f"psd_{_brk}", bufs=1, space="PSUM"))
        gcn_pools[_brk] = (_bp, _lp, _sp, _dp)

    def gcn_branch(brkey, n, w, ar_out, rs_out, atp_d, ics_sb, ico_sb, lidx,
                   clamp, wc_bf):
        njt = n // 128
        ncnk = n // 512
        packJ = max(512 // w, 1)      # j-tiles per S-pack (big 4, small 2)
        npk = njt // packJ
        if True:
            bp, lp, sp, dpup = gcn_pools[brkey]
            atp_sb = bp.tile([128, njt * w], bf16, tag="atp")
            nc.sync.dma_start(
                atp_sb[:].rearrange("p (j c) -> p j c", c=w),
                atp_d[:].rearrange("(j p) c -> p j c", p=128))

            # L0 input: H = G * (1/colsum), own slice from AR tail
            ht = bp.tile([128, n], f32, tag="hcur0")
            hto = bp.tile([128, w], f32, tag="hto0")
            with tc.tile_pool(name=f"psr_{brkey}", bufs=2, space="PSUM") as psr, \
                 tc.tile_pool(name=f"icsp_{brkey}", bufs=1) as icsp:
                g_sb = icsp.tile([128, n + w], f32, tag="g_sb")
                nc.sync.dma_start(g_sb[:, 0:n], ar_out[:])
                nc.sync.dma_start(g_sb[:, n:n + w], rs_out[:])
                for cnk in range(ncnk):
                    pr = psr.tile([128, 512], f32)
                    nc.tensor.matmul(pr[:], ones_k1[:],
                                     ics_sb[:, ts(cnk, 512)],
                                     start=True, stop=True)
                    nc.vector.tensor_tensor(
                        ht[:, ts(cnk, 512)], g_sb[:, ts(cnk, 512)], pr[:],
                        op=ALU.mult)
                pro = psr.tile([128, w], f32)
                nc.tensor.matmul(pro[:], ones_k1[:], ico_sb[:],
                                 start=True, stop=True)
                nc.vector.tensor_tensor(hto[:], g_sb[:, n:n + w], pro[:],
                                        op=ALU.mult)
                if dbg and brkey == "b":
                    nc.sync.dma_start(dbg["g1"][:], g_sb[:])
                    nc.sync.dma_start(dbg["hto_b"][:], hto[:])

            for li2, wli in enumerate(lidx):
                fo = FOS[wli]
                wl = wl_sb[wli]
                last = (li2 == 1)
                c0 = 256 + fo
                thWT = wl[:, 0:256]
                oWT = wl[:, 256:256 + fo]
                bng = wl[:, c0:c0 + 1]
                bnb = wl[:, c0 + 1:c0 + 2]
                thb = [wl[:, c0 + 2:c0 + 3], wl[:, c0 + 3:c0 + 4]]
                ob_col = wl[0:fo, c0 + 4:c0 + 5]
                ob_rep = wl[:, c0 + 5:c0 + 5 + fo]

                if True:
                    # --- P_bn: batchnorm + hbn + hx (+own) ---
                    s1 = sp.tile([128, 1], f32, tag="s1")
                    nc.vector.reduce_sum(out=s1[:], in_=ht[:], axis=AX)
                    s2p = sp.tile([128, ncnk], f32, tag="s2p")
                    sqs = sp.tile([128, 512], bf16, tag="sqscratch")
                    for cnk in range(ncnk):
                        nc.scalar.activation(
                            sqs[:], ht[:, ts(cnk, 512)], AF.Square,
                            accum_out=s2p[:, cnk:cnk + 1])
                    s2 = sp.tile([128, 1], f32, tag="s2")
                    nc.vector.reduce_sum(out=s2[:], in_=s2p[:], axis=AX)
                    m = sp.tile([128, 1], f32, tag="m")
                    nc.vector.tensor_scalar_mul(m[:], s1[:], 1.0 / n)
                    v = sp.tile([128, 1], f32, tag="v")
                    nc.vector.tensor_scalar_mul(v[:], s2[:], 1.0 / n)
                    m2 = sp.tile([128, 1], f32, tag="m2")
                    nc.vector.tensor_tensor(m2[:], m[:], m[:], op=ALU.mult)
                    nc.vector.tensor_tensor(v[:], v[:], m2[:], op=ALU.subtract)
                    sd = sp.tile([128, 1], f32, tag="sd")
                    nc.scalar.activation(sd[:], v[:], AF.Sqrt, bias=eps_c[:])
                    isd = sp.tile([128, 1], f32, tag="isd")
                    nc.vector.reciprocal(isd[:], sd[:])
                    kk = sp.tile([128, 1], f32, tag="kk")
                    nc.vector.tensor_tensor(kk[:], bng, isd[:], op=ALU.mult)
                    b2 = sp.tile([128, 1], f32, tag="b2")
                    nc.vector.tensor_tensor(b2[:], m[:], kk[:], op=ALU.mult)
                    nc.vector.tensor_tensor(b2[:], bnb, b2[:], op=ALU.subtract)
                    hbn = lp.tile([128, n], bf16, tag="hbn")
                    nc.vector.tensor_scalar(hbn[:], ht[:], kk[:], b2[:],
                                            op0=ALU.mult, op1=ALU.add)
                    hbno = lp.tile([128, w], bf16, tag="hbno")
                    nc.vector.tensor_scalar(hbno[:], hto[:], kk[:], b2[:],
                                            op0=ALU.mult, op1=ALU.add)
                    thWT_bf = lp.tile([128, 256], bf16, tag="thWT_bf")
                    nc.vector.tensor_copy(thWT_bf[:], thWT)
                    oWT_bf = lp.tile([128, fo], bf16, tag=f"oWT_bf{li2}")
                    nc.vector.tensor_copy(oWT_bf[:], oWT)

                    # Hx.T full (lhsT side) + own columns (rhs side)
                    hx = [lp.tile([128, n], bf16, tag=f"hx{k}",
                                  name=f"hx{k}") for k in range(2)]
                    hxo = [lp.tile([128, w], bf16, tag=f"hxo{k}",
                                   name=f"hxo{k}") for k in range(2)]
                    with tc.tile_pool(name=f"psx_{brkey}{li2}", bufs=2,
                                      space="PSUM") as psx:
                        for k in range(2):
                            for cnk in range(ncnk):
                                px = psx.tile([128, 512], f32)
                                nc.tensor.matmul(
                                    px[:], thWT_bf[:, ts(k, 128)],
                                    hbn[:, ts(cnk, 512)],
                                    start=True, stop=True)
                                nc.vector.tensor_scalar_add(
                                    hx[k][:, ts(cnk, 512)], px[:], thb[k])
                            pxo = psx.tile([128, w], f32)
                            nc.tensor.matmul(pxo[:], thWT_bf[:, ts(k, 128)],
                                             hbno[:], start=True, stop=True)
                            nc.vector.tensor_scalar_add(hxo[k][:], pxo[:],
                                                        thb[k])
                    yield

                    # --- P_s: S packs -> sigmoid -> t ; d rowsums; V; pus ---
                    ttp = lp.tile([128, njt * w], bf16, tag="ttp")
                    dpu = dpup.tile([128, 2 * w], f32, tag="dpu")
                    dpsum = dpu[:, 0:w]
                    pus = dpu[0:fo, w:2 * w]
                    with tc.tile_pool(name=f"pss_{brkey}{li2}", bufs=2,
                                      space="PSUM") as pssb:
                        for pk in range(npk):
                            px = pssb.tile([128, packJ * w], f32, tag="spack")
                            for jj in range(packJ):
                                j = pk * packJ + jj
                                nc.tensor.matmul(px[:, ts(jj, w)],
                                                 hx[0][:, ts(j, 128)],
                                                 hxo[0][:],
                                                 start=True, stop=False)
                                nc.tensor.matmul(px[:, ts(jj, w)],
                                                 hx[1][:, ts(j, 128)],
                                                 hxo[1][:],
                                                 start=False, stop=True)
                            sbl = sp.tile([128, packJ * w], bf16, tag="sblk")
                            nc.scalar.activation(sbl[:], px[:], AF.Sigmoid)
                            tsl = ttp[:, pk * packJ * w:(pk + 1) * packJ * w]
                            asl = atp_sb[:, pk * packJ * w:(pk + 1) * packJ * w]
                            if clamp:
                                nc.vector.scalar_tensor_tensor(
                                    tsl, sbl[:], CLAMP, asl,
                                    op0=ALU.max, op1=ALU.mult)
                            else:
                                nc.vector.tensor_tensor(tsl, sbl[:], asl,
                                                        op=ALU.mult)

                    for j in range(njt):
                        nc.tensor.matmul(dpsum, ones_bf[:],
                                         ttp[:, ts(j, w)],
                                         start=(j == 0), stop=(j == njt - 1))
                    if dbg and brkey == "b" and li2 == 0:
                        ttf = lp.tile([128, njt * w], f32, tag="ttf")
                        nc.vector.tensor_copy(ttf[:], ttp[:])
                        nc.sync.dma_start(dbg["tt_b0"][:], ttf[:])
                    drow = lp.tile([1, w], f32, tag="drow")
                    nc.vector.tensor_copy(drow[:], dpu[0:1, 0:w])
                    agd_in, agd_out = agd_bufs[(brkey, li2)]
                    nc.sync.dma_start(agd_in[:], drow[:])
                    allgather(agd_in, agd_out)

                    # V_j = H@oW + ob for all j (stats base), pus for own cols
                    stats = lp.tile([128, njt * fo], bf16, tag=f"stats{li2}")
                    nc.tensor.matmul(pus, oWT_bf[:], hbno[:],
                                     start=True, stop=True)
                    with tc.tile_pool(name=f"psh_{brkey}{li2}", bufs=1,
                                      space="PSUM") as psh:
                        for j in range(njt):
                            ph = psh.tile([128, fo], f32)
                            nc.tensor.matmul(ph[:], hbn[:, ts(j, 128)],
                                             oWT_bf[:], start=True, stop=True)
                            nc.vector.tensor_tensor(stats[:, ts(j, fo)],
                                                    ph[:], ob_rep, op=ALU.add)
                    yield

                    # --- P_d: rsqrt(d), scale, out matmul, epilogue ---
                    prcs = lp.tile([128, w], f32, tag="prcs")
                    nc.scalar.activation(prcs[:], dpsum, AF.Sqrt,
                                         bias=one_col[:])
                    prc = lp.tile([128, w], f32, tag="prc")
                    nc.vector.reciprocal(prc[:], prcs[:])
                    dcol = lp.tile([128, njt], f32, tag="dcol")
                    rsqs = lp.tile([njt, 128], f32, tag="rsqs")
                    rsqn = lp.tile([njt, 128], f32, tag="rsqn")
                    rsqi = lp.tile([njt, 128], f32, tag="rsqi")
                    nc.sync.dma_start(rsqi[:], agd_out[:])
                    if dbg and brkey == "s" and li2 == 0:
                        nc.sync.dma_start(dbg["d_s0"][:], rsqi[:])
                    nc.scalar.activation(rsqs[:], rsqi[:], AF.Sqrt,
                                         bias=one_col[0:njt, :])
                    nc.vector.reciprocal(rsqn[:], rsqs[:])
                    with tc.tile_pool(name=f"pst_{brkey}{li2}", bufs=1,
                                      space="PSUM") as pst:
                        pt = pst.tile([128, njt], f32, tag="dt")
                        nc.tensor.transpose(pt[:], rsqn[:],
                                            ident16[0:njt, 0:njt])
                        nc.vector.tensor_copy(dcol[:], pt[:])
                    for j in range(njt):
                        nc.vector.tensor_scalar_mul(stats[:, ts(j, fo)],
                                                    stats[:, ts(j, fo)],
                                                    dcol[:, j:j + 1])
                    z1 = lp.tile([fo, w], f32, tag=f"z1{li2}")
                    nc.vector.scalar_tensor_tensor(
                        z1[:], pus, ob_col, prc[0:fo, :],
                        op0=ALU.add, op1=ALU.mult)
                    with tc.tile_pool(name=f"pso_{brkey}{li2}", bufs=1,
                                      space="PSUM") as pso:
                        po = pso.tile([fo, w], f32, tag="po")
                        for j in range(njt):
                            nc.tensor.matmul(po[:], stats[:, ts(j, fo)],
                                             ttp[:, ts(j, w)],
                                             start=(j == 0),
                                             stop=(j == njt - 1))
                        s2t = lp.tile([fo, w], f32, tag=f"s2t{li2}")
                        nc.vector.tensor_tensor(s2t[:], po[:], z1[:],
                                                op=ALU.add)
                    s3t = lp.tile([fo, w], f32, tag=f"s3t{li2}")
                    nc.vector.tensor_tensor(s3t[:], s2t[:], prc[0:fo, :],
                                            op=ALU.mult)
                    if not last:
                        hnext = bp.tile([128, w], f32, tag="hto1")
                        nc.vector.scalar_tensor_tensor(
                            hnext[0:fo, :], s3t[:], 0.01, s3t[:],
                            op0=ALU.mult, op1=ALU.max)
                        agh_in, agh_out = agh_bufs[brkey]
                        nc.sync.dma_start(agh_in[0:fo, :], hnext[0:fo, :])
                        allgather(agh_in, agh_out)
                    else:
                        hnb = lp.tile([fo, w], bf16, tag="hnb")
                        nc.vector.scalar_tensor_tensor(
                            hnb[:], s3t[:], 0.01, s3t[:],
                            op0=ALU.mult, op1=ALU.max)
                        agrp_in, agrp_out = agrp_bufs[brkey]
                        with tc.tile_pool(name=f"psq_{brkey}", bufs=1,
                                          space="PSUM") as psq:
                            for i in range(w // 128):
                                pr = psq.tile([128, 32], f32)
                                nc.tensor.matmul(pr[:], hnb[:, ts(i, 128)],
                                                 wc_bf[:], start=True,
                                                 stop=True)
                                rpo = lp.tile([128, 32], f32, tag="rpo")
                                nc.vector.tensor_copy(rpo[:], pr[:])
                                nc.sync.dma_start(
                                    agrp_in[i * 128:(i + 1) * 128, :], rpo[:])
                        allgather(agrp_in, agrp_out)
                yield

                # --- P_re: reload full H for next layer ---
                if not last:
                    agh_in, agh_out = agh_bufs[brkey]
                    htn = bp.tile([fo, n], f32, tag=f"hfull{li2}")
                    nc.sync.dma_start(
                        htn[:].rearrange("f (r c) -> f r c", c=w),
                        agh_out[:].rearrange("(r f) c -> f r c", f=128))
                    if dbg and brkey == "b":
                        nc.sync.dma_start(dbg["h1"][:], htn[:])
                    ht = htn
                    hto = hnext

    gb = gcn_branch("b", nb, wb, ar1_out, rs1_out, atpb, icsb_sb, icob_sb,
                    [0, 1], True, wcb_bf)
    gs = gcn_branch("s", ns, ws, ar2_out, rs2_out, atps, icss_sb, icos_sb,
                    [2, 3], False, wcs_bf)
    # drive: big L0 fully first, then trigger AR2/RS2, then alternate
    for _ in range(3):
        next(gb)
    trigger_ar2()
    gb_done = gs_done = False
    while not (gb_done and gs_done):
        if not gs_done:
            try:
                next(gs)
            except StopIteration:
                gs_done = True
        if not gb_done:
            try:
                next(gb)
            except StopIteration:
                gb_done = True
    gcnctx.close()

    # ---- rp stationaries from AllGathered node shards ----
    rp1 = gwork.tile([128, (nb // 128) * 32], bf16, tag="rp1")
    rp1f = gwork.tile([128, (nb // 128) * 32], f32, tag="rp1f")
    nc.sync.dma_start(
        rp1f[:].rearrange("p (j c) -> p j c", c=32),
        agrp_bufs["b"][1][:].rearrange("(j p) c -> p j c", p=128))
    nc.vector.tensor_copy(rp1[:], rp1f[:])
    if dbg:
        nc.sync.dma_start(dbg["rp1"][:], rp1f[:])
    rp2 = gwork.tile([128, (ns // 128) * 32], bf16, tag="rp2")
    rp2f = gwork.tile([128, (ns // 128) * 32], f32, tag="rp2f")
    nc.sync.dma_start(
        rp2f[:].rearrange("p (j c) -> p j c", c=32),
        agrp_bufs["s"][1][:].rearrange("(j p) c -> p j c", p=128))
    nc.vector.tensor_copy(rp2[:], rp2f[:])

    # ---- pass 2 + epilogue ----
    nrc = GRP // 512
    CH = 512
    with tc.tile_pool(name="ytp", bufs=2) as ytp, \
         tc.tile_pool(name="ps_z", bufs=1, space="PSUM") as ps_z, \
         tc.tile_pool(name="ps_yw", bufs=2, space="PSUM") as ps_yw, \
         tc.tile_pool(name="ps_tp", bufs=2, space="PSUM") as ps_tp, \
         tc.tile_pool(name="epil", bufs=4) as ep:
        for gidx in range(NGRP):
            ytt = ytp.tile([65, GRP], bf16, tag="ytt")
            nc.sync.dma_start(ytt[:], yte[:, gidx * GRP:(gidx + 1) * GRP])
            pz = ps_z.tile([32, GRP], f32, tag="pz", name=f"pz{gidx}")
            ci = gidx * CPG
            nbc = nb // 128 // NJC

            def after_consume(c):
                # chunk c's consumers are emitted; its slot reuser (c+PF_BUFS)
                # may now be emitted without clobbering un-consumed data
                nonlocal next_chunk
                while next_chunk < len(chunk_order) and \
                        next_chunk <= c + PF_BUFS:
                    emit_chunk(*chunk_order[next_chunk])
                    next_chunk += 1

            for j in range(nb // 128):
                tq = pf_tiles[ci + j // NJC][:, ts(j % NJC, GRP)]
                for rc in range(nrc):
                    nc.tensor.matmul(pz[:, ts(rc, CH)], rp1[:, ts(j, 32)],
                                     tq[:, ts(rc, CH)],
                                     start=(j == 0), stop=False)
                if (j + 1) % NJC == 0:
                    after_consume(ci + j // NJC)
            for j in range(ns // 128):
                tq = pf_tiles[ci + nbc + j // NJC][:, ts(j % NJC, GRP)]
                for rc in range(nrc):
                    nc.tensor.matmul(pz[:, ts(rc, CH)], rp2[:, ts(j, 32)],
                                     tq[:, ts(rc, CH)],
                                     start=False, stop=(j == ns // 128 - 1))
                if (j + 1) % NJC == 0:
                    after_consume(ci + nbc + j // NJC)
            for rc in range(nrc):
                base = gidx * GRP + rc * CH
                pyw = ps_yw.tile([16, CH], f32)
                nc.tensor.matmul(pyw[:], w128_sb[:], ytt[:, ts(rc, CH)],
                                 start=True, stop=True)
                yws = ep.tile([16, CH], f32, tag="yws")
                nc.scalar.activation(yws[:], pyw[:], AF.Copy)
                tri = ep.tile([32, CH], f32, tag="tri")
                nc.scalar.activation(tri[:], pz[:, ts(rc, CH)], AF.Copy)
                nc.vector.tensor_tensor(tri[0:16, :], pz[0:16, ts(rc, CH)],
                                        yws[:], op=ALU.add)
                for s in range(CH // 128):
                    ptr = ps_tp.tile([128, 32], f32)
                    nc.tensor.transpose(ptr[:], tri[:, ts(s, 128)], ident32[:])
                    e = ep.tile([128, 16], f32, tag="e")
                    ssum = ep.tile([128, 1], f32, tag="ssum")
                    nc.scalar.activation(e[:], ptr[:, 0:16], AF.Exp,
                                         accum_out=ssum[:])
                    rcp = ep.tile([128, 1], f32, tag="rcp")
                    nc.vector.reciprocal(rcp[:], ssum[:])
                    yot = ep.tile([128, 16], f32, tag="yot")
                    nc.vector.tensor_scalar_mul(yot[:], e[:], rcp[:])
                    lot = ep.tile([128, 16], f32, tag="lot")
                    nc.scalar.activation(lot[:], ptr[:, 16:32], AF.Square)
                    nc.sync.dma_start(
                        yo[base + s * 128:base + (s + 1) * 128, :], yot[:])
                    nc.sync.dma_start(
                        lo[base + s * 128:base + (s + 1) * 128, :], lot[:])


def build(rows=HW // NCORES, nb=NB, ns=NS, ncores=NCORES):
    from contextlib import ExitStack
    import concourse.bacc as bacc
    import concourse.tile as tile

    nc = bacc.Bacc("TRN2", target_bir_lowering=False, debug=False,
                   enable_asserts=True, num_devices=ncores)
    with tile.TileContext(nc) as tc:
        with ExitStack() as ctx:
            _emit(nc, tc, ctx, rows, nb, ns, ncores)
    nc.compile()
    return nc


# --------------------------------------------------------------------------
# host wrapper
# --------------------------------------------------------------------------

def prep_inputs(rows, nb, ns, ncores,
                x, y, Q, A, Qsmall, Asmall,
                b0_bng, b0_bnb, b0_thW, b0_thb, b0_oW, b0_ob,
                b1_bng, b1_bnb, b1_thW, b1_thb, b1_oW, b1_ob,
                s0_bng, s0_bnb, s0_thW, s0_thb, s0_oW, s0_ob,
                s1_bng, s1_bnb, s1_thW, s1_thb, s1_oW, s1_ob,
                lin128_W, lin128_b, lin64_W, lin64_b, sigma2):
    f = np.float32
    hw = rows * ncores
    wb, ws = nb // ncores, ns // ncores
    flat = np.ascontiguousarray(np.asarray(x, f).reshape(hw, -1))
    Q = np.asarray(Q, f)
    Qs = np.asarray(Qsmall, f)
    y = np.asarray(y, f)

    icsb = (1.0 / Q.astype(BF16).astype(f).sum(axis=0))[None, :].astype(BF16)
    icss = (1.0 / Qs.astype(BF16).astype(f).sum(axis=0))[None, :].astype(BF16)

    def wl_pack(thW, thb, oW, ob, bng, bnb):
        fo = np.asarray(oW).shape[0]
        w = np.zeros((128, 256 + 2 * fo + 5), f)
        w[:, 0:256] = np.asarray(thW, f).T
        w[:, 256:256 + fo] = np.asarray(oW, f).T
        c0 = 256 + fo
        w[:, c0] = np.asarray(bng, f)
        w[:, c0 + 1] = np.asarray(bnb, f)
        w[:, c0 + 2] = np.asarray(thb, f)[0:128]
        w[:, c0 + 3] = np.asarray(thb, f)[128:256]
        w[0:fo, c0 + 4] = np.asarray(ob, f)
        w[:, c0 + 5:c0 + 5 + fo] = np.asarray(ob, f)[None, :]
        return w

    wl = [
        wl_pack(b0_thW, b0_thb, b0_oW, b0_ob, b0_bng, b0_bnb),
        wl_pack(b1_thW, b1_thb, b1_oW, b1_ob, b1_bng, b1_bnb),
        wl_pack(s0_thW, s0_thb, s0_oW, s0_ob, s0_bng, s0_bnb),
        wl_pack(s1_thW, s1_thb, s1_oW, s1_ob, s1_bng, s1_bnb),
    ]

    sig = float(np.asarray(sigma2).reshape(-1)[0])
    W128 = np.asarray(lin128_W, f)
    W64 = np.asarray(lin64_W, f)
    misc = np.zeros((64, 64), f)
    misc[:, 0:16] = sig * W128[:, :64].T
    misc[:, 16:32] = W64.T
    misc[:, 32:48] = (1.0 - sig) * W128[:, :64].T
    misc[:, 48:64] = -W64.T

    w128e = np.zeros((65, 16), f)
    w128e[0:64, :] = W128[:, 64:].T
    w128e[64, :] = np.asarray(lin128_b, f)
    w128e = w128e.astype(BF16)

    at_b = np.ascontiguousarray(np.asarray(A, f).T).astype(BF16)
    ast_b = np.ascontiguousarray(np.asarray(Asmall, f).T).astype(BF16)

    in_maps = []
    for c in range(ncores):
        r0, r1 = c * rows, (c + 1) * rows
        qsh = Q[r0:r1]
        qssh = Qs[r0:r1]
        yte = np.ones((65, rows), f)
        yte[0:64, :] = y[r0:r1].T
        m = {
            "xs": flat[r0:r1].astype(BF16),
            "q": qsh.astype(BF16),
            "qs": qssh.astype(BF16),
            "qbt": np.ascontiguousarray(qsh.T).astype(BF16),
            "qst": np.ascontiguousarray(qssh.T).astype(BF16),
            "atpb": np.ascontiguousarray(at_b[:, c * wb:(c + 1) * wb]),
            "atps": np.ascontiguousarray(ast_b[:, c * ws:(c + 1) * ws]),
            "yte": yte.astype(BF16),
            "icsb": icsb,
            "icss": icss,
            "icob": np.ascontiguousarray(icsb[:, c * wb:(c + 1) * wb]),
            "icos": np.ascontiguousarray(icss[:, c * ws:(c + 1) * ws]),
            "w128e": w128e,
            "misc": misc,
        }
        for i in range(4):
            m[f"wl{i}"] = wl[i]
        in_maps.append(m)
    return in_maps


_cache = {}
_last_results = None


def _ensure_ntff_hook():
    """Register the axon NTFF profile hook if the image's antenv lacks it."""
    import sys, types, ctypes, contextlib
    try:
        from antenv.axon_hooks import get_axon_ntff_profile_hook  # noqa: F401
        return True
    except ImportError:
        pass
    so_path = "/opt/axon/libaxon_pjrt.so"
    if not os.path.exists(so_path):
        return False
    lib = ctypes.CDLL(so_path)
    if not hasattr(lib, "axon_start_nrt_profile"):
        return False
    lib.axon_start_nrt_profile.argtypes = [ctypes.POINTER(ctypes.c_int64),
                                           ctypes.c_size_t]
    lib.axon_start_nrt_profile.restype = ctypes.c_int64
    lib.axon_stop_nrt_profile.argtypes = [ctypes.c_char_p]
    lib.axon_stop_nrt_profile.restype = ctypes.c_int64

    @contextlib.contextmanager
    def _hook(output_dir, device_ids):
        import jax
        jax.devices()
        if device_ids:
            ids = (ctypes.c_int64 * len(device_ids))(*device_ids)
            rc = lib.axon_start_nrt_profile(ids, len(device_ids))
        else:
            rc = lib.axon_start_nrt_profile(None, 0)
        if rc != 0:
            raise RuntimeError(f"axon_start_nrt_profile rc={rc}")
        try:
            yield
        finally:
            n = lib.axon_stop_nrt_profile(str(output_dir).encode())
            print(f"profile: {n} file(s) written to {output_dir}",
                  file=sys.stderr)

    mod = types.ModuleType("antenv.axon_hooks")
    holder = [_hook]
    mod.get_axon_ntff_profile_hook = lambda: holder[0]
    mod.set_axon_ntff_profile_hook = lambda h: holder.__setitem__(0, h)
    sys.modules["antenv.axon_hooks"] = mod
    import antenv
    antenv.axon_hooks = mod
    return True


def kernel(**inputs):
    global _last_results
    if "nc" not in _cache:
        _cache["nc"] = build()
    nc = _cache["nc"]
    rows = HW // NCORES
    in_maps = prep_inputs(rows, NB, NS, NCORES, **inputs)
    from concourse.bass_utils import run_bass_kernel_spmd
    trace = bool(os.environ.get("KERNEL_TRACE")) and _ensure_ntff_hook()
    res = run_bass_kernel_spmd(nc, in_maps, core_ids=list(range(NCORES)),
                               trace=trace)
    _last_results = res
    Y = np.concatenate([np.asarray(r["yo"]) for r in res.results], axis=0)
    L = np.concatenate([np.asarray(r["lo"]) for r in res.results], axis=0)
    return Y, L
